# revision 18
# baseline (speedup 1.0000x reference)
"""Trainium2 Bass kernel for a single-head transformer layer (dense_transformer).

Reference math (fp32, unscaled single-head attention):
    Q = src@Wq+bq; K = src@Wk+bk; V = src@Wv+bv
    attn = softmax(Q@K^T) @ V @ Wo + bo
    x  = LN(src + attn)*g1 + be1
    out = LN(x + relu(x@W1+b1)@W2 + b2)*g2 + be2

Sharding: 8 cores = 4 batches x 2 sequence halves. Each core computes its
1024 query rows against the full 2048-token context of its batch (K/V work
duplicated 2x; no collectives). Host slices inputs / concatenates outputs.
srcT is column-PERMUTED per core so the core's own query half occupies
columns 0..1023 (attention is permutation-invariant over context order);
Q projections are computed from srcT chunks 0-1 directly.

Host-side folds: softmax rows sum to one, so the V bias contributes
bv@Wo to every attention output row; it is folded into bo_eff = bv@Wo+bo
and V is projected without bias. bq/bk/b1 are pre-laid-out as [128, t]
(partition-major) so their DMAs are contiguous per partition.

Per-core kernel strategy (activations kept transposed so every matmul
consumes natural-layout weights; all matmuls in float32r = tf32-like).
Context chunks are processed in PAIRS so every Wq/Wk/Wv tile DMA is
shared by two 512-token chunks (halves weight traffic):
    per pair (c0, c1):
        pr==0: qT[e, q] = Wq.T @ srcT_{c0,c1}  (+bq along partitions, ACT)
        kc[e,kc]  = Wk.T @ srcT_c   (+bk, ACT)
        vc[kc,e]  = srcT_c.T @ Wv   (no bias)
        attend(c0); attend(c1):
          pc[kc,q]  = exp(kc.T @ qT)   (no max-subtraction; |logit| < ~70)
          aT[e,q]  += vc.T @ pc        (accumulated in SBUF, kept resident)
          sums[1,q]+= ones.T @ pc      (PSUM, accumulated across chunks)
    O[q,eo] = (aT.T @ Wo) * (1/sums)[q] + bo_eff ; x = LN(O + srcq)
    xT via PE transposes
    FF in f-chunk PAIRS (one FF2 PSUM group spans 8 f-tiles; the b2 bias
    enters the final group as a rank-1 ones^T@b2 matmul):
      hTc[f,q] = relu(W1c.T @ xT + b1) ; x_sb += [hTc0;hTc1].T @ [W2c0;W2c1]
    out = LN(x + ff)  (folded per q-tile into the last FF pair)
"""

import os
import numpy as np
from contextlib import ExitStack

import concourse.bacc as bacc
import concourse.tile as tile
from concourse import mybir
from concourse.masks import make_identity

P = 128
E = 1024          # embed
F = 4096          # dff
S = 2048          # context length per batch
NQ = 1024         # query rows per core
ET = E // P       # 8
FT = F // P       # 32
QS = NQ // 512    # 2 query slices of 512
KCH = 512         # k-chunk size
NCH = S // KCH    # 4 chunks
KT = KCH // P     # 4 k-tiles per chunk
FCH = 4           # f-tiles per FF chunk (512 f-columns)
f32 = mybir.dt.float32
f32r = mybir.dt.float32r
f8 = mybir.dt.float8e4
EPS = 1e-5
W1SC = 32.0       # host pre-scale of W1 into fp8 range (power of 2, exact)
W2SC = 64.0       # host pre-scale of W2 into fp8 range

USE_F32R = os.environ.get("KBENCH_F32R", "1") != "0"
SUB = mybir.AluOpType.subtract
MULT = mybir.AluOpType.mult
ADD = mybir.AluOpType.add
COPY = mybir.ActivationFunctionType.Copy
IDENT = mybir.ActivationFunctionType.Identity


def _mm(ap):
    """Bitcast matmul operands/producers to float32r (4x PE throughput at
    N>=256). The BIR verifier requires every fp32r matmul operand to be
    *produced* as fp32r, so the same bitcast is applied to the producing
    DMA (both sides) or ACT/DVE eviction output."""
    return ap.bitcast(f32r) if USE_F32R else ap


def _pt(ap_2d):
    """[ (t p), n ] DRAM view -> [p, t, n] for partition tiling."""
    return ap_2d.rearrange("(t p) n -> p t n", p=P)


def build_program():
    nc = bacc.Bacc("TRN2", target_bir_lowering=False, debug=False, num_devices=8)

    srcT = nc.dram_tensor("srcT", [E, S], f32, kind="ExternalInput").ap()
    srcq = nc.dram_tensor("srcq", [NQ, E], f32, kind="ExternalInput").ap()
    Wq = nc.dram_tensor("Wq", [E, E], f32, kind="ExternalInput").ap()
    Wk = nc.dram_tensor("Wk", [E, E], f32, kind="ExternalInput").ap()
    Wv = nc.dram_tensor("Wv", [E, E], f32, kind="ExternalInput").ap()
    Wo = nc.dram_tensor("Wo", [E, E], f32, kind="ExternalInput").ap()
    # fp8, host pre-scaled by W1SC/W2SC (compensated in relu-scale / drain)
    W1 = nc.dram_tensor("W1", [E, F], f8, kind="ExternalInput").ap()
    W2 = nc.dram_tensor("W2", [F, E], f8, kind="ExternalInput").ap()
    # host pre-laid-out [p, t]: element i at [i % 128, i // 128]
    bqp = nc.dram_tensor("bqp", [P, ET], f32, kind="ExternalInput").ap()
    bkp = nc.dram_tensor("bkp", [P, ET], f32, kind="ExternalInput").ap()
    b1p = nc.dram_tensor("b1p", [P, FT], f32, kind="ExternalInput").ap()
    bo = nc.dram_tensor("bo", [E], f32, kind="ExternalInput").ap()  # = bv@Wo+bo
    g1 = nc.dram_tensor("g1", [E], f32, kind="ExternalInput").ap()
    be1 = nc.dram_tensor("be1", [E], f32, kind="ExternalInput").ap()
    g2 = nc.dram_tensor("g2", [E], f32, kind="ExternalInput").ap()
    be2 = nc.dram_tensor("be2", [E], f32, kind="ExternalInput").ap()
    out = nc.dram_tensor("out", [NQ, E], f32, kind="ExternalOutput").ap()

    with tile.TileContext(nc) as tc, ExitStack() as ctx:
        consts = ctx.enter_context(tc.tile_pool(name="consts", bufs=1))

        # created up-front, DMA'd after the first critical-path loads
        bq_sb = consts.tile([P, ET], f32)
        bk_sb = consts.tile([P, ET], f32)
        b1_sb = consts.tile([P, FT], f32)

        # free-dim vectors broadcast across all partitions; loaded at first
        # use (phase 3+), from whatever pool is passed
        def bcast(pool, vec, n, cast=False):
            t = pool.tile([P, n], f32, tag=f"bc_{vec.tensor.name}")
            if cast:
                nc.sync.dma_start(out=_mm(t), in_=_mm(vec.partition_broadcast(P)))
            else:
                nc.sync.dma_start(out=t, in_=vec.partition_broadcast(P))
            return t

        ones0 = consts.tile([P, 1], f32)
        nc.vector.memset(ones0, 1.0)
        ones_sb = consts.tile([P, 1], f32)
        nc.vector.tensor_copy(out=_mm(ones_sb), in_=ones0)
        eps_sb = consts.tile([P, 1], f32)
        nc.vector.memset(eps_sb, EPS)
        invw2 = consts.tile([P, 1], f32)
        nc.vector.memset(invw2, 1.0 / W2SC)

        lnp = ctx.enter_context(tc.tile_pool(name="lnp", bufs=4))

        def layernorm_inplace(t, g_bc, be_bc):
            """t: [P, E] SBUF tile; LN along free dim, then *g + be.
            out = ((t - mu) * g) * rstd + be  via two fused STT ops."""
            stats = lnp.tile([P, 2, 6], f32, tag="stats")
            for sg in range(2):
                nc.vector.bn_stats(out=stats[:, sg, :], in_=t[:, sg * 512:(sg + 1) * 512])
            mv = lnp.tile([P, 2], f32, tag="mv")
            nc.vector.bn_aggr(out=mv, in_=stats)
            rstd = lnp.tile([P, 1], f32, tag="rstd")
            nc.scalar.activation(out=rstd, in_=mv[:, 1:2],
                                 func=mybir.ActivationFunctionType.Sqrt,
                                 bias=eps_sb, scale=1.0)
            nc.vector.reciprocal(out=rstd, in_=rstd)
            nc.vector.scalar_tensor_tensor(out=t, in0=t, scalar=mv[:, 0:1],
                                           in1=g_bc, op0=SUB, op1=MULT)
            nc.vector.scalar_tensor_tensor(out=t, in0=t, scalar=rstd,
                                           in1=be_bc, op0=MULT, op1=ADD)

        # aT persists across phases 2-3 in SBUF (no DRAM round-trip); on
        # the right-side stack so it can be freed before phase 4
        stA = ctx.enter_context(ExitStack())
        aT_pool = stA.enter_context(tc.tile_pool(name="aT_pool", bufs=1,
                                                 side="right"))
        aT = aT_pool.tile([P, ET, NQ], f32)

        with ExitStack() as stQA:
            # ------------- Phase 1+2: QKV projections + attention -------------
            qT_pool = stQA.enter_context(tc.tile_pool(name="qT_pool", bufs=1))
            qT = qT_pool.tile([P, ET, NQ], f32)

            st_pool = stQA.enter_context(tc.tile_pool(name="st", bufs=2))
            pc_pool = stQA.enter_context(tc.tile_pool(name="pcp", bufs=1))
            kc_pool = stQA.enter_context(tc.tile_pool(name="kcp", bufs=2))
            vc_pool = stQA.enter_context(tc.tile_pool(name="vcp", bufs=2))
            wk_pool = stQA.enter_context(tc.tile_pool(name="wk", bufs=2))
            wv_pool = stQA.enter_context(tc.tile_pool(name="wv", bufs=2))
            ps_kv = stQA.enter_context(tc.tile_pool(name="ps_kv", bufs=2, space="PSUM"))
            ps_s = stQA.enter_context(tc.tile_pool(name="ps_s", bufs=2, space="PSUM"))
            ps_a = stQA.enter_context(tc.tile_pool(name="ps_a", bufs=2, space="PSUM"))
            ps_sum = stQA.enter_context(tc.tile_pool(name="ps_sum", bufs=1, space="PSUM"))

            sums = []
            for qs in range(QS):
                sums_t = ps_sum.tile([1, 512], f32, tag=f"sums{qs}", name=f"sums{qs}")
                sums.append(sums_t)

            def attend(cc, kc, vc, first, last):
                """S^T -> exp -> sums and aT accumulation for chunk cc."""
                pc = pc_pool.tile([P, KT, NQ], f32, tag="pc", name=f"pc{cc}")
                for kt in range(KT):
                    for qs in range(QS):
                        ps = ps_s.tile([P, 512], f32, tag="ps")
                        for e_t in range(ET):
                            nc.tensor.matmul(ps, _mm(kc[:, e_t, kt * P:(kt + 1) * P]),
                                             _mm(qT[:, e_t, qs * 512:(qs + 1) * 512]),
                                             start=(e_t == 0), stop=(e_t == ET - 1))
                        nc.scalar.activation(out=_mm(pc[:, kt, qs * 512:(qs + 1) * 512]),
                                             in_=ps,
                                             func=mybir.ActivationFunctionType.Exp)
                        nc.tensor.matmul(sums[qs], _mm(ones_sb),
                                         _mm(pc[:, kt, qs * 512:(qs + 1) * 512]),
                                         start=(first and kt == 0),
                                         stop=(last and kt == KT - 1))
                # aT += vc.T @ pc
                for qs in range(QS):
                    for e_t in range(ET):
                        ps = ps_a.tile([P, 512], f32, tag="ps")
                        for kt in range(KT):
                            nc.tensor.matmul(ps, _mm(vc[:, kt, e_t * P:(e_t + 1) * P]),
                                             _mm(pc[:, kt, qs * 512:(qs + 1) * 512]),
                                             start=(kt == 0), stop=(kt == KT - 1))
                        dst = aT[:, e_t, qs * 512:(qs + 1) * 512]
                        if first:
                            nc.vector.tensor_copy(out=_mm(dst), in_=ps)
                        else:
                            nc.vector.tensor_add(out=_mm(dst), in0=dst, in1=ps)

            for pr in range(NCH // 2):
                c0, c1 = 2 * pr, 2 * pr + 1
                sts = []
                wq_first = None
                if pr == 0:
                    # very first weight tile ahead of the context DMAs so
                    # the PE can start as early as possible
                    wq_first = wk_pool.tile([P, ET, P], f32, tag="wk", name="wq_e0")
                    nc.sync.dma_start(out=_mm(wq_first),
                                      in_=_mm(_pt(Wq)[:, :, 0:P]))
                for c in (c0, c1):
                    st_c = st_pool.tile([P, ET, KCH], f32, tag="st", name=f"st{c}")
                    sts.append(st_c)
                    nsplit = 4 if c == 0 else 2
                    step = ET // nsplit
                    for h in range(nsplit):
                        nc.sync.dma_start(
                            out=_mm(st_c[:, h * step:(h + 1) * step, :]),
                            in_=_mm(_pt(srcT)[:, h * step:(h + 1) * step,
                                              c * KCH:(c + 1) * KCH]))
                    if c == 0:
                        # small bias tables: after the critical-path DMAs
                        nc.sync.dma_start(out=bq_sb, in_=bqp)
                        nc.sync.dma_start(out=bk_sb, in_=bkp)
                        nc.sync.dma_start(out=b1_sb, in_=b1p)

                if pr == 0:
                    # Q projection; chunks 0-1 ARE the core's query rows
                    # (qs = chunk index). One Wq tile serves both chunks.
                    for e_t in range(ET):
                        if e_t == 0:
                            wq_t = wq_first
                        else:
                            wq_t = wk_pool.tile([P, ET, P], f32, tag="wk",
                                                name=f"wq_e{e_t}")
                            nc.sync.dma_start(
                                out=_mm(wq_t),
                                in_=_mm(_pt(Wq)[:, :, e_t * P:(e_t + 1) * P]))
                        for qs in range(QS):
                            ps = ps_kv.tile([P, 512], f32, tag="ps")
                            for d_t in range(ET):
                                nc.tensor.matmul(ps, _mm(wq_t[:, d_t, :]),
                                                 _mm(sts[qs][:, d_t, :]),
                                                 start=(d_t == 0), stop=(d_t == ET - 1))
                            nc.scalar.activation(
                                out=_mm(qT[:, e_t, qs * 512:(qs + 1) * 512]),
                                in_=ps, func=IDENT,
                                bias=bq_sb[:, e_t:e_t + 1], scale=1.0)

                # K^T chunks [e, kc]; one Wk tile serves both chunks
                kcp = [kc_pool.tile([P, ET, KCH], f32, tag="kc", name=f"kc{c}")
                       for c in (c0, c1)]
                for e_t in range(ET):
                    wk_t = wk_pool.tile([P, ET, P], f32, tag="wk", name=f"wk{pr}_{e_t}")
                    nc.sync.dma_start(out=_mm(wk_t),
                                      in_=_mm(_pt(Wk)[:, :, e_t * P:(e_t + 1) * P]))
                    for j in range(2):
                        ps = ps_kv.tile([P, KCH], f32, tag="ps")
                        for d_t in range(ET):
                            nc.tensor.matmul(ps, _mm(wk_t[:, d_t, :]),
                                             _mm(sts[j][:, d_t, :]),
                                             start=(d_t == 0), stop=(d_t == ET - 1))
                        nc.scalar.activation(out=_mm(kcp[j][:, e_t, :]), in_=ps,
                                             func=IDENT,
                                             bias=bk_sb[:, e_t:e_t + 1], scale=1.0)

                # V chunks [kc, e] (no bias: bv folded into bo_eff on host)
                vcp = [vc_pool.tile([P, KT, E], f32, tag="vc", name=f"vc{c}")
                       for c in (c0, c1)]
                for es in range(4):
                    wv_t = wv_pool.tile([P, ET, 256], f32, tag="wv", name=f"wv{pr}_{es}")
                    nc.sync.dma_start(out=_mm(wv_t),
                                      in_=_mm(_pt(Wv)[:, :, es * 256:(es + 1) * 256]))
                    for j in range(2):
                        for kt in range(KT):
                            ps = ps_kv.tile([P, 512], f32, tag="ps")
                            pv = ps[:, 0:256]
                            for d_t in range(ET):
                                nc.tensor.matmul(
                                    pv, _mm(sts[j][:, d_t, kt * P:(kt + 1) * P]),
                                    _mm(wv_t[:, d_t, :]),
                                    start=(d_t == 0), stop=(d_t == ET - 1))
                            nc.vector.tensor_copy(
                                out=_mm(vcp[j][:, kt, es * 256:(es + 1) * 256]), in_=pv)

                attend(c0, kcp[0], vcp[0], first=(pr == 0), last=False)
                attend(c1, kcp[1], vcp[1], first=False, last=(pr == NCH // 2 - 1))

            # softmax denominators: spread sums[1, q] across partitions
            # via K=1 matmuls (1-partition DMAs fail NEFF load)
            sums_sb = consts.tile([1, NQ], f32)
            for qs in range(QS):
                nc.vector.tensor_copy(out=sums_sb[:, qs * 512:(qs + 1) * 512],
                                      in_=sums[qs])
            one_sp = consts.tile([1, 1], f32)
            nc.vector.memset(one_sp, 1.0)
            rsum = consts.tile([P, ET], f32)
            for t in range(ET):
                pst = ps_kv.tile([P, 1], f32, tag="ps", name=f"spread{t}")
                nc.tensor.matmul(pst, sums_sb[0:1, t * P:(t + 1) * P], one_sp,
                                 start=True, stop=True)
                nc.vector.tensor_copy(out=rsum[:, t:t + 1], in_=pst)
            nc.vector.reciprocal(out=rsum, in_=rsum)

        # qT + phase-2 pools freed; aT stays resident in SBUF

        # ------------- Phase 3: O, residual, LN1, transpose -------------
        x_pool = ctx.enter_context(tc.tile_pool(name="x_pool", bufs=1))
        xT_pool = ctx.enter_context(tc.tile_pool(name="xT_pool", bufs=1))
        x_sb = x_pool.tile([P, ET, E], f32)   # [q(8x128), e]
        xT = xT_pool.tile([P, ET, NQ], f8)    # [e, q] fp8 for DoubleRow FF1

        bc_pool = ctx.enter_context(tc.tile_pool(name="bc_pool", bufs=1))
        bo_bc = bcast(bc_pool, bo, E)
        g1_bc = bcast(bc_pool, g1, E)
        be1_bc = bcast(bc_pool, be1, E)
        ident = consts.tile([P, P], f32)
        make_identity(nc, ident)

        with ExitStack() as ph3:
            wo_pool = ph3.enter_context(tc.tile_pool(name="wo", bufs=1))
            sq2_pool = ph3.enter_context(tc.tile_pool(name="sq2", bufs=2))
            ps_o = ph3.enter_context(tc.tile_pool(name="ps_o", bufs=4, space="PSUM"))
            ps_t = ph3.enter_context(tc.tile_pool(name="ps_t", bufs=4, space="PSUM"))

            wo_sb = wo_pool.tile([P, ET, E], f32)
            for e_t in range(ET):
                nc.sync.dma_start(out=_mm(wo_sb[:, e_t, :]),
                                  in_=_mm(_pt(Wo)[:, e_t, :]))

            # all O-projection matmuls first (PE runs dense), DVE chains +
            # transposes follow per q-tile and overlap the remaining O MMs
            sqs = []
            for q_t in range(ET):
                sq = sq2_pool.tile([P, E], f32, tag="sq", bufs=ET, name=f"sq{q_t}")
                sqs.append(sq)
                nc.sync.dma_start(out=sq, in_=srcq[q_t * P:(q_t + 1) * P, :])
                for eo in range(2):
                    ps = ps_o.tile([P, 512], f32, tag="ps")
                    for e_t in range(ET):
                        nc.tensor.matmul(ps, _mm(aT[:, e_t, q_t * P:(q_t + 1) * P]),
                                         _mm(wo_sb[:, e_t, eo * 512:(eo + 1) * 512]),
                                         start=(e_t == 0), stop=(e_t == ET - 1))
                    # x = O*rsum + bo_eff, fused
                    nc.vector.scalar_tensor_tensor(
                        out=x_sb[:, q_t, eo * 512:(eo + 1) * 512],
                        in0=ps, scalar=rsum[:, q_t:q_t + 1],
                        in1=bo_bc[:, eo * 512:(eo + 1) * 512],
                        op0=MULT, op1=ADD)
            for q_t in range(ET):
                xt_row = x_sb[:, q_t, :]
                nc.vector.tensor_add(out=xt_row, in0=xt_row, in1=sqs[q_t])
                layernorm_inplace(xt_row, g1_bc, be1_bc)
                for e_t in range(ET):
                    pst = ps_t.tile([P, P], f32, tag="ps")
                    nc.tensor.transpose(pst, x_sb[:, q_t, e_t * P:(e_t + 1) * P], ident)
                    nc.scalar.activation(out=xT[:, e_t, q_t * P:(q_t + 1) * P],
                                         in_=pst, func=COPY)

        stA.close()  # aT freed

        # ------------- Phase 4: feedforward + LN2 -------------
        # f-chunk PAIRS: one FF2 PSUM group spans 8 f-tiles (half the
        # drains); b2 enters the last group as a rank-1 ones^T (x) b2
        # matmul; LN2 + output DMA folded per q-tile into the last pair
        g2_bc = bcast(bc_pool, g2, E)
        be2_bc = bcast(bc_pool, be2, E)
        with ExitStack() as ph4:
            w1_pool = ph4.enter_context(tc.tile_pool(name="w1p", bufs=2))
            w2_pool = ph4.enter_context(tc.tile_pool(name="w2p", bufs=3))
            hc_pool = ph4.enter_context(tc.tile_pool(name="hc", bufs=2))
            ps_h = ph4.enter_context(tc.tile_pool(name="ps_h", bufs=3, space="PSUM"))
            ps_f = ph4.enter_context(tc.tile_pool(name="ps_f", bufs=5, space="PSUM"))

            DR = mybir.MatmulPerfMode.DoubleRow
            NPAIR = FT // FCH // 2
            for fp in range(NPAIR):
                last = fp == NPAIR - 1
                hts, w2s = [], []
                for j in range(2):
                    fc = 2 * fp + j
                    w1c = w1_pool.tile([P, ET, FCH * P], f8, tag="w1", name=f"w1c{fc}")
                    nc.sync.dma_start(
                        out=w1c,
                        in_=_pt(W1)[:, :, fc * FCH * P:(fc + 1) * FCH * P])
                    hTc = hc_pool.tile([P, FCH, NQ], f8, tag="hc", name=f"hc{fc}")
                    hts.append(hTc)
                    for fl in range(FCH):
                        f_t = fc * FCH + fl
                        for qs in range(QS):
                            ps = ps_h.tile([P, 512], f32, tag="ps")
                            for ep in range(ET // 2):
                                nc.tensor.matmul(
                                    ps, w1c[:, 2 * ep:2 * ep + 2, fl * P:(fl + 1) * P],
                                    xT[:, 2 * ep:2 * ep + 2, qs * 512:(qs + 1) * 512],
                                    start=(ep == 0), stop=(ep == ET // 2 - 1),
                                    perf_mode=DR)
                            # h = relu(z + b1), z = psum/W1SC
                            nc.scalar.activation(
                                out=hTc[:, fl, qs * 512:(qs + 1) * 512],
                                in_=ps, func=mybir.ActivationFunctionType.Relu,
                                bias=b1_sb[:, f_t:f_t + 1], scale=1.0 / W1SC)

                    w2c = w2_pool.tile([P, FCH, E], f8, tag="w2", name=f"w2c{fc}")
                    w2s.append(w2c)
                    nc.sync.dma_start(out=w2c,
                                      in_=_pt(W2)[:, fc * FCH:(fc + 1) * FCH, :])

                for q_t in range(ET):
                    for eo in range(2):
                        ps = ps_f.tile([P, 512], f32, tag="ps")
                        for j in range(2):
                            for fh in range(FCH // 2):
                                nc.tensor.matmul(
                                    ps,
                                    hts[j][:, 2 * fh:2 * fh + 2, q_t * P:(q_t + 1) * P],
                                    w2s[j][:, 2 * fh:2 * fh + 2, eo * 512:(eo + 1) * 512],
                                    start=(j == 0 and fh == 0),
                                    stop=(j == 1 and fh == FCH // 2 - 1),
                                    perf_mode=DR)
                        dst = x_sb[:, q_t, eo * 512:(eo + 1) * 512]
                        # x += psum/W2SC  (fused scale+add); b2 is folded
                        # into be1 on the host (x carries it already)
                        nc.vector.scalar_tensor_tensor(
                            out=dst, in0=ps, scalar=invw2, in1=dst,
                            op0=MULT, op1=ADD)
                    if last:
                        row = x_sb[:, q_t, :]
                        layernorm_inplace(row, g2_bc, be2_bc)
                        nc.sync.dma_start(out=out[q_t * P:(q_t + 1) * P, :], in_=row)

    nc.compile()
    return nc


_NC_CACHE = None


def make_in_maps(inputs):
    import ml_dtypes

    src = np.ascontiguousarray(np.asarray(inputs["src"], dtype=np.float32))
    f = lambda n: np.asarray(inputs[n], dtype=np.float32)
    shared = {n: np.ascontiguousarray(f(n))
              for n in ["Wq", "Wk", "Wv", "Wo", "g1", "g2", "be2"]}
    # b2 is added to x before LN2; x = LN1(.)*g1 + be1, so fold b2 into be1
    shared["be1"] = np.ascontiguousarray(f("be1") + f("b2"))
    # FF weights: fp8 e4m3, pre-scaled into fp8-normal range (clip to the
    # TRN e4m3 max of +-240; power-of-2 scales are exact to invert)
    e4 = lambda a: np.clip(a, -240.0, 240.0).astype(ml_dtypes.float8_e4m3fn)
    shared["W1"] = np.ascontiguousarray(e4(f("W1") * W1SC))
    shared["W2"] = np.ascontiguousarray(e4(f("W2") * W2SC))
    # partition-major bias layouts (element i at [i % 128, i // 128])
    shared["bqp"] = np.ascontiguousarray(f("bq").reshape(ET, P).T)
    shared["bkp"] = np.ascontiguousarray(f("bk").reshape(ET, P).T)
    shared["b1p"] = np.ascontiguousarray(f("b1").reshape(FT, P).T)
    # softmax rows sum to 1 -> V-bias contributes bv@Wo to every row
    shared["bo"] = np.ascontiguousarray(f("bv") @ shared["Wo"] + f("bo"))

    in_maps = []
    for core in range(8):
        b, h = core // 2, core % 2
        src_b = src[b]                        # [2048, 1024]
        # permute context so this core's query half is columns 0..1023
        perm = np.concatenate([src_b[h * NQ:(h + 1) * NQ, :],
                               src_b[(1 - h) * NQ:(2 - h) * NQ, :]])
        srcT = np.ascontiguousarray(perm.T)   # [1024, 2048]
        srcq = np.ascontiguousarray(src_b[h * NQ:(h + 1) * NQ, :])
        in_maps.append({"srcT": srcT, "srcq": srcq, **shared})
    return in_maps


def gather_out(results):
    out = np.empty((4, S, E), np.float32)
    for core in range(8):
        b, h = core // 2, core % 2
        out[b, h * NQ:(h + 1) * NQ, :] = results[core]["out"]
    return out


def kernel(**inputs):
    global _NC_CACHE
    from concourse.bass_utils import run_bass_kernel_spmd

    in_maps = make_in_maps(inputs)
    if _NC_CACHE is None:
        _NC_CACHE = build_program()
    res = run_bass_kernel_spmd(_NC_CACHE, in_maps, list(range(8)))
    return gather_out(res.results)


if __name__ == "__main__":
    nc = build_program()
    print("build + compile OK")


# revision 19
# speedup vs baseline: 1.0078x; 1.0078x over previous
"""Trainium2 Bass kernel for a single-head transformer layer (dense_transformer).

Reference math (fp32, unscaled single-head attention):
    Q = src@Wq+bq; K = src@Wk+bk; V = src@Wv+bv
    attn = softmax(Q@K^T) @ V @ Wo + bo
    x  = LN(src + attn)*g1 + be1
    out = LN(x + relu(x@W1+b1)@W2 + b2)*g2 + be2

Sharding: 8 cores = 4 batches x 2 sequence halves. Each core computes its
1024 query rows against the full 2048-token context of its batch (K/V work
duplicated 2x; no collectives). Host slices inputs / concatenates outputs.
srcT is column-PERMUTED per core so the core's own query half occupies
columns 0..1023 (attention is permutation-invariant over context order);
Q projections are computed from srcT chunks 0-1 directly.

Host-side folds: softmax rows sum to one, so the V bias contributes
bv@Wo to every attention output row; it is folded into bo_eff = bv@Wo+bo
and V is projected without bias. bq/bk/b1 are pre-laid-out as [128, t]
(partition-major) so their DMAs are contiguous per partition.

Per-core kernel strategy (activations kept transposed so every matmul
consumes natural-layout weights; all matmuls in float32r = tf32-like).
Context chunks are processed in PAIRS so every Wq/Wk/Wv tile DMA is
shared by two 512-token chunks (halves weight traffic):
    per pair (c0, c1):
        pr==0: qT[e, q] = Wq.T @ srcT_{c0,c1}  (+bq along partitions, ACT)
        kc[e,kc]  = Wk.T @ srcT_c   (+bk, ACT)
        vc[kc,e]  = srcT_c.T @ Wv   (no bias)
        attend(c0); attend(c1):
          pc[kc,q]  = exp(kc.T @ qT)   (no max-subtraction; |logit| < ~70)
          aT[e,q]  += vc.T @ pc        (accumulated in SBUF, kept resident)
          sums[1,q]+= ones.T @ pc      (PSUM, accumulated across chunks)
    O[q,eo] = (aT.T @ Wo) * (1/sums)[q] + bo_eff ; x = LN(O + srcq)
    xT via PE transposes
    FF in f-chunk PAIRS (one FF2 PSUM group spans 8 f-tiles; the b2 bias
    enters the final group as a rank-1 ones^T@b2 matmul):
      hTc[f,q] = relu(W1c.T @ xT + b1) ; x_sb += [hTc0;hTc1].T @ [W2c0;W2c1]
    out = LN(x + ff)  (folded per q-tile into the last FF pair)
"""

import os
import numpy as np
from contextlib import ExitStack

import concourse.bacc as bacc
import concourse.tile as tile
from concourse import mybir
from concourse.masks import make_identity

P = 128
E = 1024          # embed
F = 4096          # dff
S = 2048          # context length per batch
NQ = 1024         # query rows per core
ET = E // P       # 8
FT = F // P       # 32
QS = NQ // 512    # 2 query slices of 512
KCH = 512         # k-chunk size
NCH = S // KCH    # 4 chunks
KT = KCH // P     # 4 k-tiles per chunk
FCH = 4           # f-tiles per FF chunk (512 f-columns)
f32 = mybir.dt.float32
f32r = mybir.dt.float32r
f8 = mybir.dt.float8e4
EPS = 1e-5
W1SC = 32.0       # host pre-scale of W1 into fp8 range (power of 2, exact)
W2SC = 64.0       # host pre-scale of W2 into fp8 range

USE_F32R = os.environ.get("KBENCH_F32R", "1") != "0"
SUB = mybir.AluOpType.subtract
MULT = mybir.AluOpType.mult
ADD = mybir.AluOpType.add
COPY = mybir.ActivationFunctionType.Copy
IDENT = mybir.ActivationFunctionType.Identity


def _mm(ap):
    """Bitcast matmul operands/producers to float32r (4x PE throughput at
    N>=256). The BIR verifier requires every fp32r matmul operand to be
    *produced* as fp32r, so the same bitcast is applied to the producing
    DMA (both sides) or ACT/DVE eviction output."""
    return ap.bitcast(f32r) if USE_F32R else ap


def _pt(ap_2d):
    """[ (t p), n ] DRAM view -> [p, t, n] for partition tiling."""
    return ap_2d.rearrange("(t p) n -> p t n", p=P)


def build_program():
    nc = bacc.Bacc("TRN2", target_bir_lowering=False, debug=False, num_devices=8)

    srcT = nc.dram_tensor("srcT", [E, S], f32, kind="ExternalInput").ap()
    srcq = nc.dram_tensor("srcq", [NQ, E], f32, kind="ExternalInput").ap()
    Wq = nc.dram_tensor("Wq", [E, E], f32, kind="ExternalInput").ap()
    Wk = nc.dram_tensor("Wk", [E, E], f32, kind="ExternalInput").ap()
    Wv = nc.dram_tensor("Wv", [E, E], f32, kind="ExternalInput").ap()
    Wo = nc.dram_tensor("Wo", [E, E], f32, kind="ExternalInput").ap()
    # fp8, host pre-scaled by W1SC/W2SC (compensated in relu-scale / drain)
    W1 = nc.dram_tensor("W1", [E, F], f8, kind="ExternalInput").ap()
    W2 = nc.dram_tensor("W2", [F, E], f8, kind="ExternalInput").ap()
    # host pre-laid-out [p, t]: element i at [i % 128, i // 128]
    bqp = nc.dram_tensor("bqp", [P, ET], f32, kind="ExternalInput").ap()
    bkp = nc.dram_tensor("bkp", [P, ET], f32, kind="ExternalInput").ap()
    b1p = nc.dram_tensor("b1p", [P, FT], f32, kind="ExternalInput").ap()
    bo = nc.dram_tensor("bo", [E], f32, kind="ExternalInput").ap()  # = bv@Wo+bo
    g1 = nc.dram_tensor("g1", [E], f32, kind="ExternalInput").ap()
    be1 = nc.dram_tensor("be1", [E], f32, kind="ExternalInput").ap()
    g2 = nc.dram_tensor("g2", [E], f32, kind="ExternalInput").ap()
    be2 = nc.dram_tensor("be2", [E], f32, kind="ExternalInput").ap()
    out = nc.dram_tensor("out", [NQ, E], f32, kind="ExternalOutput").ap()

    with tile.TileContext(nc) as tc, ExitStack() as ctx:
        consts = ctx.enter_context(tc.tile_pool(name="consts", bufs=1))

        # created up-front, DMA'd after the first critical-path loads
        bq_sb = consts.tile([P, ET], f32)
        bk_sb = consts.tile([P, ET], f32)
        b1_sb = consts.tile([P, FT], f32)

        # free-dim vectors broadcast across all partitions; loaded at first
        # use (phase 3+), from whatever pool is passed
        def bcast(pool, vec, n, cast=False):
            t = pool.tile([P, n], f32, tag=f"bc_{vec.tensor.name}")
            if cast:
                nc.sync.dma_start(out=_mm(t), in_=_mm(vec.partition_broadcast(P)))
            else:
                nc.sync.dma_start(out=t, in_=vec.partition_broadcast(P))
            return t

        ones0 = consts.tile([P, 1], f32)
        nc.vector.memset(ones0, 1.0)
        ones_sb = consts.tile([P, 1], f32)
        nc.vector.tensor_copy(out=_mm(ones_sb), in_=ones0)
        eps_sb = consts.tile([P, 1], f32)
        nc.vector.memset(eps_sb, EPS)
        invw2 = consts.tile([P, 1], f32)
        nc.vector.memset(invw2, 1.0 / W2SC)

        lnp = ctx.enter_context(tc.tile_pool(name="lnp", bufs=4))

        def layernorm_inplace(t, g_bc, be_bc):
            """t: [P, E] SBUF tile; LN along free dim, then *g + be.
            out = ((t - mu) * g) * rstd + be  via two fused STT ops."""
            stats = lnp.tile([P, 2, 6], f32, tag="stats")
            for sg in range(2):
                nc.vector.bn_stats(out=stats[:, sg, :], in_=t[:, sg * 512:(sg + 1) * 512])
            mv = lnp.tile([P, 2], f32, tag="mv")
            nc.vector.bn_aggr(out=mv, in_=stats)
            rstd = lnp.tile([P, 1], f32, tag="rstd")
            nc.scalar.activation(out=rstd, in_=mv[:, 1:2],
                                 func=mybir.ActivationFunctionType.Sqrt,
                                 bias=eps_sb, scale=1.0)
            nc.vector.reciprocal(out=rstd, in_=rstd)
            nc.vector.scalar_tensor_tensor(out=t, in0=t, scalar=mv[:, 0:1],
                                           in1=g_bc, op0=SUB, op1=MULT)
            nc.vector.scalar_tensor_tensor(out=t, in0=t, scalar=rstd,
                                           in1=be_bc, op0=MULT, op1=ADD)

        # aT persists across phases 2-3 in SBUF (no DRAM round-trip); on
        # the right-side stack so it can be freed before phase 4
        stA = ctx.enter_context(ExitStack())
        aT_pool = stA.enter_context(tc.tile_pool(name="aT_pool", bufs=1,
                                                 side="right"))
        aT = aT_pool.tile([P, ET, NQ], f32)

        with ExitStack() as stQA:
            # ------------- Phase 1+2: QKV projections + attention -------------
            qT_pool = stQA.enter_context(tc.tile_pool(name="qT_pool", bufs=1))
            qT = qT_pool.tile([P, ET, NQ], f32)

            st_pool = stQA.enter_context(tc.tile_pool(name="st", bufs=2))
            pc_pool = stQA.enter_context(tc.tile_pool(name="pcp", bufs=1))
            kc_pool = stQA.enter_context(tc.tile_pool(name="kcp", bufs=2))
            vc_pool = stQA.enter_context(tc.tile_pool(name="vcp", bufs=2))
            wk_pool = stQA.enter_context(tc.tile_pool(name="wk", bufs=2))
            wv_pool = stQA.enter_context(tc.tile_pool(name="wv", bufs=2))
            ps_kv = stQA.enter_context(tc.tile_pool(name="ps_kv", bufs=2, space="PSUM"))
            ps_s = stQA.enter_context(tc.tile_pool(name="ps_s", bufs=2, space="PSUM"))
            ps_a = stQA.enter_context(tc.tile_pool(name="ps_a", bufs=2, space="PSUM"))
            ps_sum = stQA.enter_context(tc.tile_pool(name="ps_sum", bufs=1, space="PSUM"))

            sums = []
            for qs in range(QS):
                sums_t = ps_sum.tile([1, 512], f32, tag=f"sums{qs}", name=f"sums{qs}")
                sums.append(sums_t)

            def attend(cc, kc, vc, first, last):
                """S^T -> exp -> sums and aT accumulation for chunk cc."""
                pc = pc_pool.tile([P, KT, NQ], f32, tag="pc", name=f"pc{cc}")
                for kt in range(KT):
                    for qs in range(QS):
                        ps = ps_s.tile([P, 512], f32, tag="ps")
                        for e_t in range(ET):
                            nc.tensor.matmul(ps, _mm(kc[:, e_t, kt * P:(kt + 1) * P]),
                                             _mm(qT[:, e_t, qs * 512:(qs + 1) * 512]),
                                             start=(e_t == 0), stop=(e_t == ET - 1))
                        nc.scalar.activation(out=_mm(pc[:, kt, qs * 512:(qs + 1) * 512]),
                                             in_=ps,
                                             func=mybir.ActivationFunctionType.Exp)
                        nc.tensor.matmul(sums[qs], _mm(ones_sb),
                                         _mm(pc[:, kt, qs * 512:(qs + 1) * 512]),
                                         start=(first and kt == 0),
                                         stop=(last and kt == KT - 1))
                # aT += vc.T @ pc
                for qs in range(QS):
                    for e_t in range(ET):
                        ps = ps_a.tile([P, 512], f32, tag="ps")
                        for kt in range(KT):
                            nc.tensor.matmul(ps, _mm(vc[:, kt, e_t * P:(e_t + 1) * P]),
                                             _mm(pc[:, kt, qs * 512:(qs + 1) * 512]),
                                             start=(kt == 0), stop=(kt == KT - 1))
                        dst = aT[:, e_t, qs * 512:(qs + 1) * 512]
                        if first:
                            nc.vector.tensor_copy(out=_mm(dst), in_=ps)
                        else:
                            nc.vector.tensor_add(out=_mm(dst), in0=dst, in1=ps)

            for pr in range(NCH // 2):
                c0, c1 = 2 * pr, 2 * pr + 1
                sts = []
                wq_first = None
                if pr == 0:
                    # very first weight tile ahead of the context DMAs so
                    # the PE can start as early as possible
                    wq_first = wk_pool.tile([P, ET, P], f32, tag="wk", name="wq_e0")
                    nc.sync.dma_start(out=_mm(wq_first),
                                      in_=_mm(_pt(Wq)[:, :, 0:P]))
                for c in (c0, c1):
                    st_c = st_pool.tile([P, ET, KCH], f32, tag="st", name=f"st{c}")
                    sts.append(st_c)
                    nsplit = 4 if c == 0 else 2
                    step = ET // nsplit
                    for h in range(nsplit):
                        nc.sync.dma_start(
                            out=_mm(st_c[:, h * step:(h + 1) * step, :]),
                            in_=_mm(_pt(srcT)[:, h * step:(h + 1) * step,
                                              c * KCH:(c + 1) * KCH]))
                    if c == 0:
                        # small bias tables: after the critical-path DMAs
                        nc.sync.dma_start(out=bq_sb, in_=bqp)
                        nc.sync.dma_start(out=bk_sb, in_=bkp)
                        nc.sync.dma_start(out=b1_sb, in_=b1p)

                if pr == 0:
                    # Q projection; chunks 0-1 ARE the core's query rows
                    # (qs = chunk index). One Wq tile serves both chunks.
                    for e_t in range(ET):
                        if e_t == 0:
                            wq_t = wq_first
                        else:
                            wq_t = wk_pool.tile([P, ET, P], f32, tag="wk",
                                                name=f"wq_e{e_t}")
                            nc.sync.dma_start(
                                out=_mm(wq_t),
                                in_=_mm(_pt(Wq)[:, :, e_t * P:(e_t + 1) * P]))
                        for qs in range(QS):
                            ps = ps_kv.tile([P, 512], f32, tag="ps")
                            for d_t in range(ET):
                                nc.tensor.matmul(ps, _mm(wq_t[:, d_t, :]),
                                                 _mm(sts[qs][:, d_t, :]),
                                                 start=(d_t == 0), stop=(d_t == ET - 1))
                            nc.scalar.activation(
                                out=_mm(qT[:, e_t, qs * 512:(qs + 1) * 512]),
                                in_=ps, func=IDENT,
                                bias=bq_sb[:, e_t:e_t + 1], scale=1.0)

                # K^T chunks [e, kc]; one Wk tile serves both chunks
                kcp = [kc_pool.tile([P, ET, KCH], f32, tag="kc", name=f"kc{c}")
                       for c in (c0, c1)]
                for e_t in range(ET):
                    wk_t = wk_pool.tile([P, ET, P], f32, tag="wk", name=f"wk{pr}_{e_t}")
                    nc.sync.dma_start(out=_mm(wk_t),
                                      in_=_mm(_pt(Wk)[:, :, e_t * P:(e_t + 1) * P]))
                    for j in range(2):
                        ps = ps_kv.tile([P, KCH], f32, tag="ps")
                        for d_t in range(ET):
                            nc.tensor.matmul(ps, _mm(wk_t[:, d_t, :]),
                                             _mm(sts[j][:, d_t, :]),
                                             start=(d_t == 0), stop=(d_t == ET - 1))
                        nc.scalar.activation(out=_mm(kcp[j][:, e_t, :]), in_=ps,
                                             func=IDENT,
                                             bias=bk_sb[:, e_t:e_t + 1], scale=1.0)

                # V chunks [kc, e] (no bias: bv folded into bo_eff on host)
                vcp = [vc_pool.tile([P, KT, E], f32, tag="vc", name=f"vc{c}")
                       for c in (c0, c1)]
                for es in range(4):
                    wv_t = wv_pool.tile([P, ET, 256], f32, tag="wv", name=f"wv{pr}_{es}")
                    nc.sync.dma_start(out=_mm(wv_t),
                                      in_=_mm(_pt(Wv)[:, :, es * 256:(es + 1) * 256]))
                    for j in range(2):
                        for kt in range(KT):
                            ps = ps_kv.tile([P, 512], f32, tag="ps")
                            pv = ps[:, 0:256]
                            for d_t in range(ET):
                                nc.tensor.matmul(
                                    pv, _mm(sts[j][:, d_t, kt * P:(kt + 1) * P]),
                                    _mm(wv_t[:, d_t, :]),
                                    start=(d_t == 0), stop=(d_t == ET - 1))
                            nc.vector.tensor_copy(
                                out=_mm(vcp[j][:, kt, es * 256:(es + 1) * 256]), in_=pv)

                attend(c0, kcp[0], vcp[0], first=(pr == 0), last=False)
                attend(c1, kcp[1], vcp[1], first=False, last=(pr == NCH // 2 - 1))

            # softmax denominators: spread sums[1, q] across partitions
            # via K=1 matmuls (1-partition DMAs fail NEFF load)
            sums_sb = consts.tile([1, NQ], f32)
            for qs in range(QS):
                nc.vector.tensor_copy(out=sums_sb[:, qs * 512:(qs + 1) * 512],
                                      in_=sums[qs])
            one_sp = consts.tile([1, 1], f32)
            nc.vector.memset(one_sp, 1.0)
            rsum = consts.tile([P, ET], f32)
            for t in range(ET):
                pst = ps_kv.tile([P, 1], f32, tag="ps", name=f"spread{t}")
                nc.tensor.matmul(pst, sums_sb[0:1, t * P:(t + 1) * P], one_sp,
                                 start=True, stop=True)
                nc.vector.tensor_copy(out=rsum[:, t:t + 1], in_=pst)
            nc.vector.reciprocal(out=rsum, in_=rsum)

        # qT + phase-2 pools freed; aT stays resident in SBUF

        # ------------- Phase 3: O, residual, LN1, transpose -------------
        x_pool = ctx.enter_context(tc.tile_pool(name="x_pool", bufs=1))
        xT_pool = ctx.enter_context(tc.tile_pool(name="xT_pool", bufs=1))
        x_sb = x_pool.tile([P, ET, E], f32)   # [q(8x128), e]
        xT = xT_pool.tile([P, ET, NQ], f8)    # [e, q] fp8 for DoubleRow FF1

        bc_pool = ctx.enter_context(tc.tile_pool(name="bc_pool", bufs=1))
        bo_bc = bcast(bc_pool, bo, E)
        g1_bc = bcast(bc_pool, g1, E)
        be1_bc = bcast(bc_pool, be1, E)
        ident = consts.tile([P, P], f32)
        make_identity(nc, ident)

        with ExitStack() as ph3:
            wo_pool = ph3.enter_context(tc.tile_pool(name="wo", bufs=1))
            sq2_pool = ph3.enter_context(tc.tile_pool(name="sq2", bufs=2))
            ps_o = ph3.enter_context(tc.tile_pool(name="ps_o", bufs=4, space="PSUM"))
            ps_t = ph3.enter_context(tc.tile_pool(name="ps_t", bufs=4, space="PSUM"))

            wo_sb = wo_pool.tile([P, ET, E], f32)
            for e_t in range(ET):
                nc.sync.dma_start(out=_mm(wo_sb[:, e_t, :]),
                                  in_=_mm(_pt(Wo)[:, e_t, :]))

            for q_t in range(ET):
                sq = sq2_pool.tile([P, E], f32, tag="sq")
                nc.sync.dma_start(out=sq, in_=srcq[q_t * P:(q_t + 1) * P, :])
                for eo in range(2):
                    ps = ps_o.tile([P, 512], f32, tag="ps")
                    for e_t in range(ET):
                        nc.tensor.matmul(ps, _mm(aT[:, e_t, q_t * P:(q_t + 1) * P]),
                                         _mm(wo_sb[:, e_t, eo * 512:(eo + 1) * 512]),
                                         start=(e_t == 0), stop=(e_t == ET - 1))
                    # x = O*rsum + bo_eff, fused
                    nc.vector.scalar_tensor_tensor(
                        out=x_sb[:, q_t, eo * 512:(eo + 1) * 512],
                        in0=ps, scalar=rsum[:, q_t:q_t + 1],
                        in1=bo_bc[:, eo * 512:(eo + 1) * 512],
                        op0=MULT, op1=ADD)
                xt_row = x_sb[:, q_t, :]
                nc.vector.tensor_add(out=xt_row, in0=xt_row, in1=sq)
                layernorm_inplace(xt_row, g1_bc, be1_bc)
                for e_t in range(ET):
                    pst = ps_t.tile([P, P], f32, tag="ps")
                    nc.tensor.transpose(pst, x_sb[:, q_t, e_t * P:(e_t + 1) * P], ident)
                    nc.scalar.activation(out=xT[:, e_t, q_t * P:(q_t + 1) * P],
                                         in_=pst, func=COPY)

        stA.close()  # aT freed

        # ------------- Phase 4: feedforward + LN2 -------------
        # f-chunk PAIRS: one FF2 PSUM group spans 8 f-tiles (half the
        # drains); b2 enters the last group as a rank-1 ones^T (x) b2
        # matmul; LN2 + output DMA folded per q-tile into the last pair
        g2_bc = bcast(bc_pool, g2, E)
        be2_bc = bcast(bc_pool, be2, E)
        with ExitStack() as ph4:
            w1_pool = ph4.enter_context(tc.tile_pool(name="w1p", bufs=2))
            w2_pool = ph4.enter_context(tc.tile_pool(name="w2p", bufs=3))
            hc_pool = ph4.enter_context(tc.tile_pool(name="hc", bufs=2))
            ps_h = ph4.enter_context(tc.tile_pool(name="ps_h", bufs=3, space="PSUM"))
            ps_f = ph4.enter_context(tc.tile_pool(name="ps_f", bufs=5, space="PSUM"))

            DR = mybir.MatmulPerfMode.DoubleRow
            NPAIR = FT // FCH // 2
            for fp in range(NPAIR):
                last = fp == NPAIR - 1
                hts, w2s = [], []
                for j in range(2):
                    fc = 2 * fp + j
                    w1c = w1_pool.tile([P, ET, FCH * P], f8, tag="w1", name=f"w1c{fc}")
                    nc.sync.dma_start(
                        out=w1c,
                        in_=_pt(W1)[:, :, fc * FCH * P:(fc + 1) * FCH * P])
                    hTc = hc_pool.tile([P, FCH, NQ], f8, tag="hc", name=f"hc{fc}")
                    hts.append(hTc)
                    for fl in range(FCH):
                        f_t = fc * FCH + fl
                        for qs in range(QS):
                            ps = ps_h.tile([P, 512], f32, tag="ps")
                            for ep in range(ET // 2):
                                nc.tensor.matmul(
                                    ps, w1c[:, 2 * ep:2 * ep + 2, fl * P:(fl + 1) * P],
                                    xT[:, 2 * ep:2 * ep + 2, qs * 512:(qs + 1) * 512],
                                    start=(ep == 0), stop=(ep == ET // 2 - 1),
                                    perf_mode=DR)
                            # h = relu(z + b1), z = psum/W1SC
                            nc.scalar.activation(
                                out=hTc[:, fl, qs * 512:(qs + 1) * 512],
                                in_=ps, func=mybir.ActivationFunctionType.Relu,
                                bias=b1_sb[:, f_t:f_t + 1], scale=1.0 / W1SC)

                    w2c = w2_pool.tile([P, FCH, E], f8, tag="w2", name=f"w2c{fc}")
                    w2s.append(w2c)
                    nc.sync.dma_start(out=w2c,
                                      in_=_pt(W2)[:, fc * FCH:(fc + 1) * FCH, :])

                for q_t in range(ET):
                    for eo in range(2):
                        ps = ps_f.tile([P, 512], f32, tag="ps")
                        for j in range(2):
                            for fh in range(FCH // 2):
                                nc.tensor.matmul(
                                    ps,
                                    hts[j][:, 2 * fh:2 * fh + 2, q_t * P:(q_t + 1) * P],
                                    w2s[j][:, 2 * fh:2 * fh + 2, eo * 512:(eo + 1) * 512],
                                    start=(j == 0 and fh == 0),
                                    stop=(j == 1 and fh == FCH // 2 - 1),
                                    perf_mode=DR)
                        dst = x_sb[:, q_t, eo * 512:(eo + 1) * 512]
                        # x += psum/W2SC  (fused scale+add); b2 is folded
                        # into be1 on the host (x carries it already)
                        nc.vector.scalar_tensor_tensor(
                            out=dst, in0=ps, scalar=invw2, in1=dst,
                            op0=MULT, op1=ADD)
                    if last:
                        row = x_sb[:, q_t, :]
                        layernorm_inplace(row, g2_bc, be2_bc)
                        nc.sync.dma_start(out=out[q_t * P:(q_t + 1) * P, :], in_=row)

    nc.compile()
    return nc


_NC_CACHE = None


def make_in_maps(inputs):
    import ml_dtypes

    src = np.ascontiguousarray(np.asarray(inputs["src"], dtype=np.float32))
    f = lambda n: np.asarray(inputs[n], dtype=np.float32)
    shared = {n: np.ascontiguousarray(f(n))
              for n in ["Wq", "Wk", "Wv", "Wo", "g1", "g2", "be2"]}
    # b2 is added to x before LN2; x = LN1(.)*g1 + be1, so fold b2 into be1
    shared["be1"] = np.ascontiguousarray(f("be1") + f("b2"))
    # FF weights: fp8 e4m3, pre-scaled into fp8-normal range (clip to the
    # TRN e4m3 max of +-240; power-of-2 scales are exact to invert)
    e4 = lambda a: np.clip(a, -240.0, 240.0).astype(ml_dtypes.float8_e4m3fn)
    shared["W1"] = np.ascontiguousarray(e4(f("W1") * W1SC))
    shared["W2"] = np.ascontiguousarray(e4(f("W2") * W2SC))
    # partition-major bias layouts (element i at [i % 128, i // 128])
    shared["bqp"] = np.ascontiguousarray(f("bq").reshape(ET, P).T)
    shared["bkp"] = np.ascontiguousarray(f("bk").reshape(ET, P).T)
    shared["b1p"] = np.ascontiguousarray(f("b1").reshape(FT, P).T)
    # softmax rows sum to 1 -> V-bias contributes bv@Wo to every row
    shared["bo"] = np.ascontiguousarray(f("bv") @ shared["Wo"] + f("bo"))

    in_maps = []
    for core in range(8):
        b, h = core // 2, core % 2
        src_b = src[b]                        # [2048, 1024]
        # permute context so this core's query half is columns 0..1023
        perm = np.concatenate([src_b[h * NQ:(h + 1) * NQ, :],
                               src_b[(1 - h) * NQ:(2 - h) * NQ, :]])
        srcT = np.ascontiguousarray(perm.T)   # [1024, 2048]
        srcq = np.ascontiguousarray(src_b[h * NQ:(h + 1) * NQ, :])
        in_maps.append({"srcT": srcT, "srcq": srcq, **shared})
    return in_maps


def gather_out(results):
    out = np.empty((4, S, E), np.float32)
    for core in range(8):
        b, h = core // 2, core % 2
        out[b, h * NQ:(h + 1) * NQ, :] = results[core]["out"]
    return out


def kernel(**inputs):
    global _NC_CACHE
    from concourse.bass_utils import run_bass_kernel_spmd

    in_maps = make_in_maps(inputs)
    if _NC_CACHE is None:
        _NC_CACHE = build_program()
    res = run_bass_kernel_spmd(_NC_CACHE, in_maps, list(range(8)))
    return gather_out(res.results)


if __name__ == "__main__":
    nc = build_program()
    print("build + compile OK")


# revision 20
# speedup vs baseline: 1.0628x; 1.0545x over previous
"""Trainium2 Bass kernel for a single-head transformer layer (dense_transformer).

Reference math (fp32, unscaled single-head attention):
    Q = src@Wq+bq; K = src@Wk+bk; V = src@Wv+bv
    attn = softmax(Q@K^T) @ V @ Wo + bo
    x  = LN(src + attn)*g1 + be1
    out = LN(x + relu(x@W1+b1)@W2 + b2)*g2 + be2

Sharding: 8 cores = 4 batches x 2 sequence halves. Each core computes its
1024 query rows against the full 2048-token context of its batch (K/V work
duplicated 2x; no collectives). Host slices inputs / concatenates outputs.
srcT is column-PERMUTED per core so the core's own query half occupies
columns 0..1023 (attention is permutation-invariant over context order);
Q projections are computed from srcT chunks 0-1 directly.

Host-side folds: softmax rows sum to one, so the V bias contributes
bv@Wo to every attention output row; it is folded into bo_eff = bv@Wo+bo
and V is projected without bias. bq/bk/b1 are pre-laid-out as [128, t]
(partition-major) so their DMAs are contiguous per partition.

Per-core kernel strategy (activations kept transposed so every matmul
consumes natural-layout weights; all matmuls in float32r = tf32-like).
Context chunks are processed in PAIRS so every Wq/Wk/Wv tile DMA is
shared by two 512-token chunks (halves weight traffic):
    per pair (c0, c1):
        pr==0: qT[e, q] = Wq.T @ srcT_{c0,c1}  (+bq along partitions, ACT)
        kc[e,kc]  = Wk.T @ srcT_c   (+bk, ACT)
        vc[kc,e]  = srcT_c.T @ Wv   (no bias)
        attend(c0); attend(c1):
          pc[kc,q]  = exp(kc.T @ qT)   (no max-subtraction; |logit| < ~70)
          aT[e,q]  += vc.T @ pc        (accumulated in SBUF, kept resident)
          sums[1,q]+= ones.T @ pc      (PSUM, accumulated across chunks)
    O[q,eo] = (aT.T @ Wo) * (1/sums)[q] + bo_eff ; x = LN(O + srcq)
    xT via PE transposes
    FF in f-chunk PAIRS (one FF2 PSUM group spans 8 f-tiles; the b2 bias
    enters the final group as a rank-1 ones^T@b2 matmul):
      hTc[f,q] = relu(W1c.T @ xT + b1) ; x_sb += [hTc0;hTc1].T @ [W2c0;W2c1]
    out = LN(x + ff)  (folded per q-tile into the last FF pair)
"""

import os
import numpy as np
from contextlib import ExitStack

import concourse.bacc as bacc
import concourse.tile as tile
from concourse import mybir
from concourse.masks import make_identity

P = 128
E = 1024          # embed
F = 4096          # dff
S = 2048          # context length per batch
NQ = 1024         # query rows per core
ET = E // P       # 8
FT = F // P       # 32
QS = NQ // 512    # 2 query slices of 512
KCH = 512         # k-chunk size
NCH = S // KCH    # 4 chunks
KT = KCH // P     # 4 k-tiles per chunk
FCH = 4           # f-tiles per FF chunk (512 f-columns)
f32 = mybir.dt.float32
f32r = mybir.dt.float32r
f8 = mybir.dt.float8e4
EPS = 1e-5
W1SC = 32.0       # host pre-scale of W1 into fp8 range (power of 2, exact)
W2SC = 64.0       # host pre-scale of W2 into fp8 range

USE_F32R = os.environ.get("KBENCH_F32R", "1") != "0"
SUB = mybir.AluOpType.subtract
MULT = mybir.AluOpType.mult
ADD = mybir.AluOpType.add
COPY = mybir.ActivationFunctionType.Copy
IDENT = mybir.ActivationFunctionType.Identity


def _mm(ap):
    """Bitcast matmul operands/producers to float32r (4x PE throughput at
    N>=256). The BIR verifier requires every fp32r matmul operand to be
    *produced* as fp32r, so the same bitcast is applied to the producing
    DMA (both sides) or ACT/DVE eviction output."""
    return ap.bitcast(f32r) if USE_F32R else ap


def _pt(ap_2d):
    """[ (t p), n ] DRAM view -> [p, t, n] for partition tiling."""
    return ap_2d.rearrange("(t p) n -> p t n", p=P)


def build_program():
    nc = bacc.Bacc("TRN2", target_bir_lowering=False, debug=False, num_devices=8)

    srcT = nc.dram_tensor("srcT", [E, S], f32, kind="ExternalInput").ap()
    srcq = nc.dram_tensor("srcq", [NQ, E], f32, kind="ExternalInput").ap()
    Wq = nc.dram_tensor("Wq", [E, E], f32, kind="ExternalInput").ap()
    Wk = nc.dram_tensor("Wk", [E, E], f32, kind="ExternalInput").ap()
    Wv = nc.dram_tensor("Wv", [E, E], f32, kind="ExternalInput").ap()
    Wo = nc.dram_tensor("Wo", [E, E], f32, kind="ExternalInput").ap()
    # fp8, host pre-scaled by W1SC/W2SC (compensated in relu-scale / drain)
    W1 = nc.dram_tensor("W1", [E, F], f8, kind="ExternalInput").ap()
    W2 = nc.dram_tensor("W2", [F, E], f8, kind="ExternalInput").ap()
    # host pre-laid-out [p, t]: element i at [i % 128, i // 128]
    bqp = nc.dram_tensor("bqp", [P, ET], f32, kind="ExternalInput").ap()
    bkp = nc.dram_tensor("bkp", [P, ET], f32, kind="ExternalInput").ap()
    b1p = nc.dram_tensor("b1p", [P, FT], f32, kind="ExternalInput").ap()
    g1 = nc.dram_tensor("g1", [E], f32, kind="ExternalInput").ap()
    be1 = nc.dram_tensor("be1", [E], f32, kind="ExternalInput").ap()
    g2 = nc.dram_tensor("g2", [E], f32, kind="ExternalInput").ap()
    be2 = nc.dram_tensor("be2", [E], f32, kind="ExternalInput").ap()
    out = nc.dram_tensor("out", [NQ, E], f32, kind="ExternalOutput").ap()

    with tile.TileContext(nc) as tc, ExitStack() as ctx:
        consts = ctx.enter_context(tc.tile_pool(name="consts", bufs=1))

        # created up-front, DMA'd after the first critical-path loads
        bq_sb = consts.tile([P, ET], f32)
        bk_sb = consts.tile([P, ET], f32)
        b1_sb = consts.tile([P, FT], f32)

        # free-dim vectors broadcast across all partitions; loaded at first
        # use (phase 3+), from whatever pool is passed
        def bcast(pool, vec, n, cast=False):
            t = pool.tile([P, n], f32, tag=f"bc_{vec.tensor.name}")
            if cast:
                nc.sync.dma_start(out=_mm(t), in_=_mm(vec.partition_broadcast(P)))
            else:
                nc.sync.dma_start(out=t, in_=vec.partition_broadcast(P))
            return t

        ones0 = consts.tile([P, 1], f32)
        nc.vector.memset(ones0, 1.0)
        ones_sb = consts.tile([P, 1], f32)
        nc.vector.tensor_copy(out=_mm(ones_sb), in_=ones0)
        eps_sb = consts.tile([P, 1], f32)
        nc.vector.memset(eps_sb, EPS)
        invw2 = consts.tile([P, 1], f32)
        nc.vector.memset(invw2, 1.0 / W2SC)

        lnp = ctx.enter_context(tc.tile_pool(name="lnp", bufs=4))

        def layernorm_inplace(t, g_bc, be_bc):
            """t: [P, E] SBUF tile; LN along free dim, then *g + be.
            out = ((t - mu) * g) * rstd + be  via two fused STT ops."""
            stats = lnp.tile([P, 2, 6], f32, tag="stats")
            for sg in range(2):
                nc.vector.bn_stats(out=stats[:, sg, :], in_=t[:, sg * 512:(sg + 1) * 512])
            mv = lnp.tile([P, 2], f32, tag="mv")
            nc.vector.bn_aggr(out=mv, in_=stats)
            rstd = lnp.tile([P, 1], f32, tag="rstd")
            nc.scalar.activation(out=rstd, in_=mv[:, 1:2],
                                 func=mybir.ActivationFunctionType.Sqrt,
                                 bias=eps_sb, scale=1.0)
            nc.vector.reciprocal(out=rstd, in_=rstd)
            nc.vector.scalar_tensor_tensor(out=t, in0=t, scalar=mv[:, 0:1],
                                           in1=g_bc, op0=SUB, op1=MULT)
            nc.vector.scalar_tensor_tensor(out=t, in0=t, scalar=rstd,
                                           in1=be_bc, op0=MULT, op1=ADD)

        # aT persists across phases 2-3 in SBUF (no DRAM round-trip); on
        # the right-side stack so it can be freed before phase 4
        stA = ctx.enter_context(ExitStack())
        aT_pool = stA.enter_context(tc.tile_pool(name="aT_pool", bufs=1,
                                                 side="right"))
        aT = aT_pool.tile([P, ET, NQ], f32)

        with ExitStack() as stQA:
            # ------------- Phase 1+2: QKV projections + attention -------------
            qT_pool = stQA.enter_context(tc.tile_pool(name="qT_pool", bufs=1))
            qT = qT_pool.tile([P, ET, NQ], f32)

            st_pool = stQA.enter_context(tc.tile_pool(name="st", bufs=2))
            kc_pool = stQA.enter_context(tc.tile_pool(name="kcp", bufs=2))
            pc_pool = stQA.enter_context(tc.tile_pool(name="pcp", bufs=1))
            vc_pool = stQA.enter_context(tc.tile_pool(name="vcp", bufs=2))
            wk_pool = stQA.enter_context(tc.tile_pool(name="wk", bufs=2))
            wv_pool = stQA.enter_context(tc.tile_pool(name="wv", bufs=2))
            ps_kv = stQA.enter_context(tc.tile_pool(name="ps_kv", bufs=2, space="PSUM"))
            ps_s = stQA.enter_context(tc.tile_pool(name="ps_s", bufs=2, space="PSUM"))
            ps_a = stQA.enter_context(tc.tile_pool(name="ps_a", bufs=2, space="PSUM"))
            ps_sum = stQA.enter_context(tc.tile_pool(name="ps_sum", bufs=1, space="PSUM"))

            sums = []
            for qs in range(QS):
                sums_t = ps_sum.tile([1, 512], f32, tag=f"sums{qs}", name=f"sums{qs}")
                sums.append(sums_t)

            def attend(cc, kc, vc, first, last):
                """S^T -> exp -> sums and aT accumulation for chunk cc."""
                pc = pc_pool.tile([P, KT, NQ], f32, tag="pc", name=f"pc{cc}")
                for kt in range(KT):
                    for qs in range(QS):
                        ps = ps_s.tile([P, 512], f32, tag="ps")
                        for e_t in range(ET):
                            nc.tensor.matmul(ps, _mm(kc[:, e_t, kt * P:(kt + 1) * P]),
                                             _mm(qT[:, e_t, qs * 512:(qs + 1) * 512]),
                                             start=(e_t == 0), stop=(e_t == ET - 1))
                        nc.scalar.activation(out=_mm(pc[:, kt, qs * 512:(qs + 1) * 512]),
                                             in_=ps,
                                             func=mybir.ActivationFunctionType.Exp)
                        nc.tensor.matmul(sums[qs], _mm(ones_sb),
                                         _mm(pc[:, kt, qs * 512:(qs + 1) * 512]),
                                         start=(first and kt == 0),
                                         stop=(last and kt == KT - 1))
                # aT += vc.T @ pc
                for qs in range(QS):
                    for e_t in range(ET):
                        ps = ps_a.tile([P, 512], f32, tag="ps")
                        for kt in range(KT):
                            nc.tensor.matmul(ps, _mm(vc[:, kt, e_t * P:(e_t + 1) * P]),
                                             _mm(pc[:, kt, qs * 512:(qs + 1) * 512]),
                                             start=(kt == 0), stop=(kt == KT - 1))
                        dst = aT[:, e_t, qs * 512:(qs + 1) * 512]
                        if first:
                            nc.vector.tensor_copy(out=_mm(dst), in_=ps)
                        else:
                            nc.vector.tensor_add(out=_mm(dst), in0=dst, in1=ps)

            for pr in range(NCH // 2):
                c0, c1 = 2 * pr, 2 * pr + 1
                sts = []
                wq_first = None
                if pr == 0:
                    # very first weight tile ahead of the context DMAs so
                    # the PE can start as early as possible
                    wq_first = wk_pool.tile([P, ET, P], f32, tag="wk", name="wq_e0")
                    nc.sync.dma_start(out=_mm(wq_first),
                                      in_=_mm(_pt(Wq)[:, :, 0:P]))
                for c in (c0, c1):
                    st_c = st_pool.tile([P, ET, KCH], f32, tag="st", name=f"st{c}")
                    sts.append(st_c)
                    nsplit = 4 if c == 0 else 2
                    step = ET // nsplit
                    for h in range(nsplit):
                        nc.sync.dma_start(
                            out=_mm(st_c[:, h * step:(h + 1) * step, :]),
                            in_=_mm(_pt(srcT)[:, h * step:(h + 1) * step,
                                              c * KCH:(c + 1) * KCH]))
                    if c == 0:
                        # small bias tables: after the critical-path DMAs
                        nc.sync.dma_start(out=bq_sb, in_=bqp)
                        nc.sync.dma_start(out=bk_sb, in_=bkp)
                        nc.sync.dma_start(out=b1_sb, in_=b1p)

                if pr == 0:
                    # Q projection; chunks 0-1 ARE the core's query rows
                    # (qs = chunk index). One Wq tile serves both chunks.
                    for e_t in range(ET):
                        if e_t == 0:
                            wq_t = wq_first
                        else:
                            wq_t = wk_pool.tile([P, ET, P], f32, tag="wk",
                                                name=f"wq_e{e_t}")
                            nc.sync.dma_start(
                                out=_mm(wq_t),
                                in_=_mm(_pt(Wq)[:, :, e_t * P:(e_t + 1) * P]))
                        for qs in range(QS):
                            ps = ps_kv.tile([P, 512], f32, tag="ps")
                            for d_t in range(ET):
                                nc.tensor.matmul(ps, _mm(wq_t[:, d_t, :]),
                                                 _mm(sts[qs][:, d_t, :]),
                                                 start=(d_t == 0), stop=(d_t == ET - 1))
                            nc.scalar.activation(
                                out=_mm(qT[:, e_t, qs * 512:(qs + 1) * 512]),
                                in_=ps, func=IDENT,
                                bias=bq_sb[:, e_t:e_t + 1], scale=1.0)

                # K^T chunks [e, kc]; one Wk tile serves both chunks
                kcp = [kc_pool.tile([P, ET, KCH], f32, tag="kc", name=f"kc{c}")
                       for c in (c0, c1)]
                for e_t in range(ET):
                    wk_t = wk_pool.tile([P, ET, P], f32, tag="wk", name=f"wk{pr}_{e_t}")
                    nc.sync.dma_start(out=_mm(wk_t),
                                      in_=_mm(_pt(Wk)[:, :, e_t * P:(e_t + 1) * P]))
                    for j in range(2):
                        ps = ps_kv.tile([P, KCH], f32, tag="ps")
                        for d_t in range(ET):
                            nc.tensor.matmul(ps, _mm(wk_t[:, d_t, :]),
                                             _mm(sts[j][:, d_t, :]),
                                             start=(d_t == 0), stop=(d_t == ET - 1))
                        nc.scalar.activation(out=_mm(kcp[j][:, e_t, :]), in_=ps,
                                             func=IDENT,
                                             bias=bk_sb[:, e_t:e_t + 1], scale=1.0)

                # V chunks [kc, e] (no bias: bv folded into bo_eff on host)
                vcp = [vc_pool.tile([P, KT, E], f32, tag="vc", name=f"vc{c}")
                       for c in (c0, c1)]
                for es in range(4):
                    wv_t = wv_pool.tile([P, ET, 256], f32, tag="wv", name=f"wv{pr}_{es}")
                    nc.sync.dma_start(out=_mm(wv_t),
                                      in_=_mm(_pt(Wv)[:, :, es * 256:(es + 1) * 256]))
                    for j in range(2):
                        for kt in range(KT):
                            ps = ps_kv.tile([P, 512], f32, tag="ps")
                            pv = ps[:, 0:256]
                            for d_t in range(ET):
                                nc.tensor.matmul(
                                    pv, _mm(sts[j][:, d_t, kt * P:(kt + 1) * P]),
                                    _mm(wv_t[:, d_t, :]),
                                    start=(d_t == 0), stop=(d_t == ET - 1))
                            nc.vector.tensor_copy(
                                out=_mm(vcp[j][:, kt, es * 256:(es + 1) * 256]), in_=pv)

                attend(c0, kcp[0], vcp[0], first=(pr == 0), last=False)
                attend(c1, kcp[1], vcp[1], first=False, last=(pr == NCH // 2 - 1))

            # softmax denominators: spread sums[1, q] across partitions
            # via K=1 matmuls (1-partition DMAs fail NEFF load)
            sums_sb = consts.tile([1, NQ], f32)
            for qs in range(QS):
                nc.vector.tensor_copy(out=sums_sb[:, qs * 512:(qs + 1) * 512],
                                      in_=sums[qs])
            one_sp = consts.tile([1, 1], f32)
            nc.vector.memset(one_sp, 1.0)
            rsum = consts.tile([P, ET], f32)
            for t in range(ET):
                pst = ps_kv.tile([P, 1], f32, tag="ps", name=f"spread{t}")
                nc.tensor.matmul(pst, sums_sb[0:1, t * P:(t + 1) * P], one_sp,
                                 start=True, stop=True)
                nc.vector.tensor_copy(out=rsum[:, t:t + 1], in_=pst)
            nc.vector.reciprocal(out=rsum, in_=rsum)

        # qT + phase-2 pools freed; aT stays resident in SBUF

        # ------------- Phase 3: O, residual, LN1, transpose -------------
        # creation order matters: each pool lands on the phase-2 ranges that
        # die earliest (x->qT, xT->st, wo->st/kc, bc/sq->kc), so their DMAs
        # overlap the tail of the attention phase
        x_pool = ctx.enter_context(tc.tile_pool(name="x_pool", bufs=1))
        xT_pool = ctx.enter_context(tc.tile_pool(name="xT_pool", bufs=1))
        x_sb = x_pool.tile([P, ET, E], f32)   # [q(8x128), e]
        xT = xT_pool.tile([P, ET, NQ], f8)    # [e, q] fp8 for DoubleRow FF1

        wo_pool = ctx.enter_context(tc.tile_pool(name="wo", bufs=1))
        bc_pool = ctx.enter_context(tc.tile_pool(name="bc_pool", bufs=1))
        sq2_pool = ctx.enter_context(tc.tile_pool(name="sq2", bufs=2))
        g1_bc = bcast(bc_pool, g1, E)
        be1_bc = bcast(bc_pool, be1, E)
        ident = consts.tile([P, P], f32)
        make_identity(nc, ident)

        with ExitStack() as ph3:
            ps_o = ph3.enter_context(tc.tile_pool(name="ps_o", bufs=4, space="PSUM"))
            ps_t = ph3.enter_context(tc.tile_pool(name="ps_t", bufs=4, space="PSUM"))

            wo_sb = wo_pool.tile([P, ET, E], f32)
            for e_t in range(ET):
                nc.sync.dma_start(out=_mm(wo_sb[:, e_t, :]),
                                  in_=_mm(_pt(Wo)[:, e_t, :]))

            for q_t in range(ET):
                sq = sq2_pool.tile([P, E], f32, tag="sq")
                nc.sync.dma_start(out=sq, in_=srcq[q_t * P:(q_t + 1) * P, :])
                for eo in range(2):
                    ps = ps_o.tile([P, 512], f32, tag="ps")
                    for e_t in range(ET):
                        nc.tensor.matmul(ps, _mm(aT[:, e_t, q_t * P:(q_t + 1) * P]),
                                         _mm(wo_sb[:, e_t, eo * 512:(eo + 1) * 512]),
                                         start=(e_t == 0), stop=(e_t == ET - 1))
                    # x = O*rsum + (srcq + bo_eff)  (bo folded into srcq on host)
                    nc.vector.scalar_tensor_tensor(
                        out=x_sb[:, q_t, eo * 512:(eo + 1) * 512],
                        in0=ps, scalar=rsum[:, q_t:q_t + 1],
                        in1=sq[:, eo * 512:(eo + 1) * 512],
                        op0=MULT, op1=ADD)
                xt_row = x_sb[:, q_t, :]
                layernorm_inplace(xt_row, g1_bc, be1_bc)
                for e_t in range(ET):
                    pst = ps_t.tile([P, P], f32, tag="ps")
                    nc.tensor.transpose(pst, x_sb[:, q_t, e_t * P:(e_t + 1) * P], ident)
                    nc.scalar.activation(out=xT[:, e_t, q_t * P:(q_t + 1) * P],
                                         in_=pst, func=COPY)

        stA.close()  # aT freed

        # ------------- Phase 4: feedforward + LN2 -------------
        # f-chunk PAIRS: one FF2 PSUM group spans 8 f-tiles (half the
        # drains); b2 enters the last group as a rank-1 ones^T (x) b2
        # matmul; LN2 + output DMA folded per q-tile into the last pair
        g2_bc = bcast(bc_pool, g2, E)
        be2_bc = bcast(bc_pool, be2, E)
        with ExitStack() as ph4:
            w1_pool = ph4.enter_context(tc.tile_pool(name="w1p", bufs=2))
            w2_pool = ph4.enter_context(tc.tile_pool(name="w2p", bufs=3))
            hc_pool = ph4.enter_context(tc.tile_pool(name="hc", bufs=2))
            ps_h = ph4.enter_context(tc.tile_pool(name="ps_h", bufs=3, space="PSUM"))
            ps_f = ph4.enter_context(tc.tile_pool(name="ps_f", bufs=5, space="PSUM"))

            DR = mybir.MatmulPerfMode.DoubleRow
            NPAIR = FT // FCH // 2
            for fp in range(NPAIR):
                last = fp == NPAIR - 1
                hts, w2s = [], []
                for j in range(2):
                    fc = 2 * fp + j
                    w1c = w1_pool.tile([P, ET, FCH * P], f8, tag="w1", name=f"w1c{fc}")
                    nc.sync.dma_start(
                        out=w1c,
                        in_=_pt(W1)[:, :, fc * FCH * P:(fc + 1) * FCH * P])
                    hTc = hc_pool.tile([P, FCH, NQ], f8, tag="hc", name=f"hc{fc}")
                    hts.append(hTc)
                    flqs = ([(fl, qs) for qs in range(QS) for fl in range(FCH)]
                            if fc == 0 else
                            [(fl, qs) for fl in range(FCH) for qs in range(QS)])
                    for fl, qs in flqs:
                        f_t = fc * FCH + fl
                        if True:
                            ps = ps_h.tile([P, 512], f32, tag="ps")
                            for ep in range(ET // 2):
                                nc.tensor.matmul(
                                    ps, w1c[:, 2 * ep:2 * ep + 2, fl * P:(fl + 1) * P],
                                    xT[:, 2 * ep:2 * ep + 2, qs * 512:(qs + 1) * 512],
                                    start=(ep == 0), stop=(ep == ET // 2 - 1),
                                    perf_mode=DR)
                            # h = relu(z + b1), z = psum/W1SC
                            nc.scalar.activation(
                                out=hTc[:, fl, qs * 512:(qs + 1) * 512],
                                in_=ps, func=mybir.ActivationFunctionType.Relu,
                                bias=b1_sb[:, f_t:f_t + 1], scale=1.0 / W1SC)

                    w2c = w2_pool.tile([P, FCH, E], f8, tag="w2", name=f"w2c{fc}")
                    w2s.append(w2c)
                    nc.sync.dma_start(out=w2c,
                                      in_=_pt(W2)[:, fc * FCH:(fc + 1) * FCH, :])

                for q_t in range(ET):
                    for eo in range(2):
                        ps = ps_f.tile([P, 512], f32, tag="ps")
                        for j in range(2):
                            for fh in range(FCH // 2):
                                nc.tensor.matmul(
                                    ps,
                                    hts[j][:, 2 * fh:2 * fh + 2, q_t * P:(q_t + 1) * P],
                                    w2s[j][:, 2 * fh:2 * fh + 2, eo * 512:(eo + 1) * 512],
                                    start=(j == 0 and fh == 0),
                                    stop=(j == 1 and fh == FCH // 2 - 1),
                                    perf_mode=DR)
                        dst = x_sb[:, q_t, eo * 512:(eo + 1) * 512]
                        # x += psum/W2SC  (fused scale+add); b2 is folded
                        # into be1 on the host (x carries it already)
                        nc.vector.scalar_tensor_tensor(
                            out=dst, in0=ps, scalar=invw2, in1=dst,
                            op0=MULT, op1=ADD)
                    if last:
                        row = x_sb[:, q_t, :]
                        layernorm_inplace(row, g2_bc, be2_bc)
                        nc.sync.dma_start(out=out[q_t * P:(q_t + 1) * P, :], in_=row)

    nc.compile()
    return nc


_NC_CACHE = None


def make_in_maps(inputs):
    import ml_dtypes

    src = np.ascontiguousarray(np.asarray(inputs["src"], dtype=np.float32))
    f = lambda n: np.asarray(inputs[n], dtype=np.float32)
    shared = {n: np.ascontiguousarray(f(n))
              for n in ["Wq", "Wk", "Wv", "Wo", "g1", "g2", "be2"]}
    # b2 is added to x before LN2; x = LN1(.)*g1 + be1, so fold b2 into be1
    shared["be1"] = np.ascontiguousarray(f("be1") + f("b2"))
    # FF weights: fp8 e4m3, pre-scaled into fp8-normal range (clip to the
    # TRN e4m3 max of +-240; power-of-2 scales are exact to invert)
    e4 = lambda a: np.clip(a, -240.0, 240.0).astype(ml_dtypes.float8_e4m3fn)
    shared["W1"] = np.ascontiguousarray(e4(f("W1") * W1SC))
    shared["W2"] = np.ascontiguousarray(e4(f("W2") * W2SC))
    # partition-major bias layouts (element i at [i % 128, i // 128])
    shared["bqp"] = np.ascontiguousarray(f("bq").reshape(ET, P).T)
    shared["bkp"] = np.ascontiguousarray(f("bk").reshape(ET, P).T)
    shared["b1p"] = np.ascontiguousarray(f("b1").reshape(FT, P).T)
    # softmax rows sum to 1 -> V-bias contributes bv@Wo to every row;
    # the whole O-bias is folded into the residual srcq on the host
    bo_eff = f("bv") @ shared["Wo"] + f("bo")

    in_maps = []
    for core in range(8):
        b, h = core // 2, core % 2
        src_b = src[b]                        # [2048, 1024]
        # permute context so this core's query half is columns 0..1023
        perm = np.concatenate([src_b[h * NQ:(h + 1) * NQ, :],
                               src_b[(1 - h) * NQ:(2 - h) * NQ, :]])
        srcT = np.ascontiguousarray(perm.T)   # [1024, 2048]
        srcq = np.ascontiguousarray(src_b[h * NQ:(h + 1) * NQ, :] + bo_eff)
        in_maps.append({"srcT": srcT, "srcq": srcq, **shared})
    return in_maps


def gather_out(results):
    out = np.empty((4, S, E), np.float32)
    for core in range(8):
        b, h = core // 2, core % 2
        out[b, h * NQ:(h + 1) * NQ, :] = results[core]["out"]
    return out


def kernel(**inputs):
    global _NC_CACHE
    from concourse.bass_utils import run_bass_kernel_spmd

    in_maps = make_in_maps(inputs)
    if _NC_CACHE is None:
        _NC_CACHE = build_program()
    res = run_bass_kernel_spmd(_NC_CACHE, in_maps, list(range(8)))
    return gather_out(res.results)


if __name__ == "__main__":
    nc = build_program()
    print("build + compile OK")


# revision 22
# speedup vs baseline: 1.0635x; 1.0007x over previous
"""Trainium2 Bass kernel for a single-head transformer layer (dense_transformer).

Reference math (fp32, unscaled single-head attention):
    Q = src@Wq+bq; K = src@Wk+bk; V = src@Wv+bv
    attn = softmax(Q@K^T) @ V @ Wo + bo
    x  = LN(src + attn)*g1 + be1
    out = LN(x + relu(x@W1+b1)@W2 + b2)*g2 + be2

Sharding: 8 cores = 4 batches x 2 sequence halves. Each core computes its
1024 query rows against the full 2048-token context of its batch (K/V work
duplicated 2x; no collectives). Host slices inputs / concatenates outputs.
srcT is column-PERMUTED per core so the core's own query half occupies
columns 0..1023 (attention is permutation-invariant over context order);
Q projections are computed from srcT chunks 0-1 directly.

Host-side folds: softmax rows sum to one, so the V bias contributes
bv@Wo to every attention output row; it is folded into bo_eff = bv@Wo+bo
and V is projected without bias. bq/bk/b1 are pre-laid-out as [128, t]
(partition-major) so their DMAs are contiguous per partition.

Per-core kernel strategy (activations kept transposed so every matmul
consumes natural-layout weights; all matmuls in float32r = tf32-like).
Context chunks are processed in PAIRS so every Wq/Wk/Wv tile DMA is
shared by two 512-token chunks (halves weight traffic):
    per pair (c0, c1):
        pr==0: qT[e, q] = Wq.T @ srcT_{c0,c1}  (+bq along partitions, ACT)
        kc[e,kc]  = Wk.T @ srcT_c   (+bk, ACT)
        vc[kc,e]  = srcT_c.T @ Wv   (no bias)
        attend(c0); attend(c1):
          pc[kc,q]  = exp(kc.T @ qT)   (no max-subtraction; |logit| < ~70)
          aT[e,q]  += vc.T @ pc        (accumulated in SBUF, kept resident)
          sums[1,q]+= ones.T @ pc      (PSUM, accumulated across chunks)
    O[q,eo] = (aT.T @ Wo) * (1/sums)[q] + bo_eff ; x = LN(O + srcq)
    xT via PE transposes
    FF in f-chunk PAIRS (one FF2 PSUM group spans 8 f-tiles; the b2 bias
    enters the final group as a rank-1 ones^T@b2 matmul):
      hTc[f,q] = relu(W1c.T @ xT + b1) ; x_sb += [hTc0;hTc1].T @ [W2c0;W2c1]
    out = LN(x + ff)  (folded per q-tile into the last FF pair)
"""

import os
import numpy as np
from contextlib import ExitStack

import concourse.bacc as bacc
import concourse.tile as tile
from concourse import mybir
from concourse.masks import make_identity

P = 128
E = 1024          # embed
F = 4096          # dff
S = 2048          # context length per batch
NQ = 1024         # query rows per core
ET = E // P       # 8
FT = F // P       # 32
QS = NQ // 512    # 2 query slices of 512
KCH = 512         # k-chunk size
NCH = S // KCH    # 4 chunks
KT = KCH // P     # 4 k-tiles per chunk
FCH = 4           # f-tiles per FF chunk (512 f-columns)
f32 = mybir.dt.float32
f32r = mybir.dt.float32r
f8 = mybir.dt.float8e4
EPS = 1e-5
W1SC = 32.0       # host pre-scale of W1 into fp8 range (power of 2, exact)
W2SC = 64.0       # host pre-scale of W2 into fp8 range

USE_F32R = os.environ.get("KBENCH_F32R", "1") != "0"
SUB = mybir.AluOpType.subtract
MULT = mybir.AluOpType.mult
ADD = mybir.AluOpType.add
COPY = mybir.ActivationFunctionType.Copy
IDENT = mybir.ActivationFunctionType.Identity


def _mm(ap):
    """Bitcast matmul operands/producers to float32r (4x PE throughput at
    N>=256). The BIR verifier requires every fp32r matmul operand to be
    *produced* as fp32r, so the same bitcast is applied to the producing
    DMA (both sides) or ACT/DVE eviction output."""
    return ap.bitcast(f32r) if USE_F32R else ap


def _pt(ap_2d):
    """[ (t p), n ] DRAM view -> [p, t, n] for partition tiling."""
    return ap_2d.rearrange("(t p) n -> p t n", p=P)


def build_program():
    nc = bacc.Bacc("TRN2", target_bir_lowering=False, debug=False, num_devices=8)

    srcT = nc.dram_tensor("srcT", [E, S], f32, kind="ExternalInput").ap()
    srcq = nc.dram_tensor("srcq", [NQ, E], f32, kind="ExternalInput").ap()
    Wq = nc.dram_tensor("Wq", [E, E], f32, kind="ExternalInput").ap()
    Wk = nc.dram_tensor("Wk", [E, E], f32, kind="ExternalInput").ap()
    Wv = nc.dram_tensor("Wv", [E, E], f32, kind="ExternalInput").ap()
    Wo = nc.dram_tensor("Wo", [E, E], f32, kind="ExternalInput").ap()
    # fp8, host pre-scaled by W1SC/W2SC (compensated in relu-scale / drain)
    W1 = nc.dram_tensor("W1", [E, F], f8, kind="ExternalInput").ap()
    W2 = nc.dram_tensor("W2", [F, E], f8, kind="ExternalInput").ap()
    # host pre-laid-out [p, t]: element i at [i % 128, i // 128]
    bqp = nc.dram_tensor("bqp", [P, ET], f32, kind="ExternalInput").ap()
    bkp = nc.dram_tensor("bkp", [P, ET], f32, kind="ExternalInput").ap()
    b1p = nc.dram_tensor("b1p", [P, FT], f32, kind="ExternalInput").ap()
    g1 = nc.dram_tensor("g1", [E], f32, kind="ExternalInput").ap()
    be1 = nc.dram_tensor("be1", [E], f32, kind="ExternalInput").ap()
    g2 = nc.dram_tensor("g2", [E], f32, kind="ExternalInput").ap()
    be2 = nc.dram_tensor("be2", [E], f32, kind="ExternalInput").ap()
    out = nc.dram_tensor("out", [NQ, E], f32, kind="ExternalOutput").ap()

    with tile.TileContext(nc) as tc, ExitStack() as ctx:
        consts = ctx.enter_context(tc.tile_pool(name="consts", bufs=1))

        # created up-front, DMA'd after the first critical-path loads
        bq_sb = consts.tile([P, ET], f32)
        bk_sb = consts.tile([P, ET], f32)
        b1_sb = consts.tile([P, FT], f32)

        # free-dim vectors broadcast across all partitions; loaded at first
        # use (phase 3+), from whatever pool is passed
        def bcast(pool, vec, n, cast=False):
            t = pool.tile([P, n], f32, tag=f"bc_{vec.tensor.name}")
            if cast:
                nc.sync.dma_start(out=_mm(t), in_=_mm(vec.partition_broadcast(P)))
            else:
                nc.sync.dma_start(out=t, in_=vec.partition_broadcast(P))
            return t

        ones0 = consts.tile([P, 1], f32)
        nc.vector.memset(ones0, 1.0)
        ones_sb = consts.tile([P, 1], f32)
        nc.vector.tensor_copy(out=_mm(ones_sb), in_=ones0)
        eps_sb = consts.tile([P, 1], f32)
        nc.vector.memset(eps_sb, EPS)
        invw2 = consts.tile([P, 1], f32)
        nc.vector.memset(invw2, 1.0 / W2SC)

        lnp = ctx.enter_context(tc.tile_pool(name="lnp", bufs=4))

        def layernorm_inplace(t, g_bc, be_bc, eng=None):
            """t: [P, E] SBUF tile; LN along free dim, then *g + be.
            out = ((t - mu) * g) * rstd + be  via two fused STT ops (on
            `eng` - gpsimd offload relieves the DVE at phase tails)."""
            eng = eng or nc.vector
            stats = lnp.tile([P, 2, 6], f32, tag="stats")
            for sg in range(2):
                nc.vector.bn_stats(out=stats[:, sg, :], in_=t[:, sg * 512:(sg + 1) * 512])
            mv = lnp.tile([P, 2], f32, tag="mv")
            nc.vector.bn_aggr(out=mv, in_=stats)
            rstd = lnp.tile([P, 1], f32, tag="rstd")
            nc.scalar.activation(out=rstd, in_=mv[:, 1:2],
                                 func=mybir.ActivationFunctionType.Sqrt,
                                 bias=eps_sb, scale=1.0)
            nc.vector.reciprocal(out=rstd, in_=rstd)
            eng.scalar_tensor_tensor(out=t, in0=t, scalar=mv[:, 0:1],
                                     in1=g_bc, op0=SUB, op1=MULT)
            eng.scalar_tensor_tensor(out=t, in0=t, scalar=rstd,
                                     in1=be_bc, op0=MULT, op1=ADD)

        # aT persists across phases 2-3 in SBUF (no DRAM round-trip); on
        # the right-side stack so it can be freed before phase 4
        stA = ctx.enter_context(ExitStack())
        aT_pool = stA.enter_context(tc.tile_pool(name="aT_pool", bufs=1,
                                                 side="right"))
        aT = aT_pool.tile([P, ET, NQ], f32)

        with ExitStack() as stQA:
            # ------------- Phase 1+2: QKV projections + attention -------------
            qT_pool = stQA.enter_context(tc.tile_pool(name="qT_pool", bufs=1))
            qT = qT_pool.tile([P, ET, NQ], f32)

            st_pool = stQA.enter_context(tc.tile_pool(name="st", bufs=2))
            kc_pool = stQA.enter_context(tc.tile_pool(name="kcp", bufs=2))
            pc_pool = stQA.enter_context(tc.tile_pool(name="pcp", bufs=1))
            vc_pool = stQA.enter_context(tc.tile_pool(name="vcp", bufs=2))
            wk_pool = stQA.enter_context(tc.tile_pool(name="wk", bufs=2))
            wv_pool = stQA.enter_context(tc.tile_pool(name="wv", bufs=2))
            ps_kv = stQA.enter_context(tc.tile_pool(name="ps_kv", bufs=2, space="PSUM"))
            ps_s = stQA.enter_context(tc.tile_pool(name="ps_s", bufs=2, space="PSUM"))
            ps_a = stQA.enter_context(tc.tile_pool(name="ps_a", bufs=2, space="PSUM"))
            ps_sum = stQA.enter_context(tc.tile_pool(name="ps_sum", bufs=1, space="PSUM"))

            sums = []
            for qs in range(QS):
                sums_t = ps_sum.tile([1, 512], f32, tag=f"sums{qs}", name=f"sums{qs}")
                sums.append(sums_t)

            def attend(cc, kc, vc, first, last):
                """S^T -> exp -> sums and aT accumulation for chunk cc."""
                pc = pc_pool.tile([P, KT, NQ], f32, tag="pc", name=f"pc{cc}")
                for kt in range(KT):
                    for qs in range(QS):
                        ps = ps_s.tile([P, 512], f32, tag="ps")
                        for e_t in range(ET):
                            nc.tensor.matmul(ps, _mm(kc[:, e_t, kt * P:(kt + 1) * P]),
                                             _mm(qT[:, e_t, qs * 512:(qs + 1) * 512]),
                                             start=(e_t == 0), stop=(e_t == ET - 1))
                        nc.scalar.activation(out=_mm(pc[:, kt, qs * 512:(qs + 1) * 512]),
                                             in_=ps,
                                             func=mybir.ActivationFunctionType.Exp)
                        nc.tensor.matmul(sums[qs], _mm(ones_sb),
                                         _mm(pc[:, kt, qs * 512:(qs + 1) * 512]),
                                         start=(first and kt == 0),
                                         stop=(last and kt == KT - 1))
                # aT += vc.T @ pc
                for qs in range(QS):
                    for e_t in range(ET):
                        ps = ps_a.tile([P, 512], f32, tag="ps")
                        for kt in range(KT):
                            nc.tensor.matmul(ps, _mm(vc[:, kt, e_t * P:(e_t + 1) * P]),
                                             _mm(pc[:, kt, qs * 512:(qs + 1) * 512]),
                                             start=(kt == 0), stop=(kt == KT - 1))
                        dst = aT[:, e_t, qs * 512:(qs + 1) * 512]
                        if first:
                            nc.vector.tensor_copy(out=_mm(dst), in_=ps)
                        else:
                            nc.vector.tensor_add(out=_mm(dst), in0=dst, in1=ps)

            for pr in range(NCH // 2):
                c0, c1 = 2 * pr, 2 * pr + 1
                sts = []
                wq_first = None
                if pr == 0:
                    # very first weight tile ahead of the context DMAs so
                    # the PE can start as early as possible
                    wq_first = wk_pool.tile([P, ET, P], f32, tag="wk", name="wq_e0")
                    nc.sync.dma_start(out=_mm(wq_first),
                                      in_=_mm(_pt(Wq)[:, :, 0:P]))
                for c in (c0, c1):
                    st_c = st_pool.tile([P, ET, KCH], f32, tag="st", name=f"st{c}")
                    sts.append(st_c)
                    nsplit = 4 if c == 0 else 2
                    step = ET // nsplit
                    for h in range(nsplit):
                        nc.sync.dma_start(
                            out=_mm(st_c[:, h * step:(h + 1) * step, :]),
                            in_=_mm(_pt(srcT)[:, h * step:(h + 1) * step,
                                              c * KCH:(c + 1) * KCH]))
                    if c == 0:
                        # small bias tables: after the critical-path DMAs
                        nc.sync.dma_start(out=bq_sb, in_=bqp)
                        nc.sync.dma_start(out=bk_sb, in_=bkp)
                        nc.sync.dma_start(out=b1_sb, in_=b1p)

                if pr == 0:
                    # Q projection; chunks 0-1 ARE the core's query rows
                    # (qs = chunk index). One Wq tile serves both chunks.
                    for e_t in range(ET):
                        if e_t == 0:
                            wq_t = wq_first
                        else:
                            wq_t = wk_pool.tile([P, ET, P], f32, tag="wk",
                                                name=f"wq_e{e_t}")
                            nc.sync.dma_start(
                                out=_mm(wq_t),
                                in_=_mm(_pt(Wq)[:, :, e_t * P:(e_t + 1) * P]))
                        for qs in range(QS):
                            ps = ps_kv.tile([P, 512], f32, tag="ps")
                            for d_t in range(ET):
                                nc.tensor.matmul(ps, _mm(wq_t[:, d_t, :]),
                                                 _mm(sts[qs][:, d_t, :]),
                                                 start=(d_t == 0), stop=(d_t == ET - 1))
                            nc.scalar.activation(
                                out=_mm(qT[:, e_t, qs * 512:(qs + 1) * 512]),
                                in_=ps, func=IDENT,
                                bias=bq_sb[:, e_t:e_t + 1], scale=1.0)

                # K^T chunks [e, kc]; one Wk tile serves both chunks
                kcp = [kc_pool.tile([P, ET, KCH], f32, tag="kc", name=f"kc{c}")
                       for c in (c0, c1)]
                for e_t in range(ET):
                    wk_t = wk_pool.tile([P, ET, P], f32, tag="wk", name=f"wk{pr}_{e_t}")
                    nc.sync.dma_start(out=_mm(wk_t),
                                      in_=_mm(_pt(Wk)[:, :, e_t * P:(e_t + 1) * P]))
                    for j in range(2):
                        ps = ps_kv.tile([P, KCH], f32, tag="ps")
                        for d_t in range(ET):
                            nc.tensor.matmul(ps, _mm(wk_t[:, d_t, :]),
                                             _mm(sts[j][:, d_t, :]),
                                             start=(d_t == 0), stop=(d_t == ET - 1))
                        nc.scalar.activation(out=_mm(kcp[j][:, e_t, :]), in_=ps,
                                             func=IDENT,
                                             bias=bk_sb[:, e_t:e_t + 1], scale=1.0)

                # V chunks [kc, e] (no bias: bv folded into bo_eff on host)
                vcp = [vc_pool.tile([P, KT, E], f32, tag="vc", name=f"vc{c}")
                       for c in (c0, c1)]
                for es in range(4):
                    wv_t = wv_pool.tile([P, ET, 256], f32, tag="wv", name=f"wv{pr}_{es}")
                    nc.sync.dma_start(out=_mm(wv_t),
                                      in_=_mm(_pt(Wv)[:, :, es * 256:(es + 1) * 256]))
                    for j in range(2):
                        for kt in range(KT):
                            ps = ps_kv.tile([P, 512], f32, tag="ps")
                            pv = ps[:, 0:256]
                            for d_t in range(ET):
                                nc.tensor.matmul(
                                    pv, _mm(sts[j][:, d_t, kt * P:(kt + 1) * P]),
                                    _mm(wv_t[:, d_t, :]),
                                    start=(d_t == 0), stop=(d_t == ET - 1))
                            nc.vector.tensor_copy(
                                out=_mm(vcp[j][:, kt, es * 256:(es + 1) * 256]), in_=pv)

                attend(c0, kcp[0], vcp[0], first=(pr == 0), last=False)
                attend(c1, kcp[1], vcp[1], first=False, last=(pr == NCH // 2 - 1))

            # softmax denominators: spread sums[1, q] across partitions
            # via K=1 matmuls (1-partition DMAs fail NEFF load)
            sums_sb = consts.tile([1, NQ], f32)
            for qs in range(QS):
                nc.vector.tensor_copy(out=sums_sb[:, qs * 512:(qs + 1) * 512],
                                      in_=sums[qs])
            one_sp = consts.tile([1, 1], f32)
            nc.vector.memset(one_sp, 1.0)
            rsum = consts.tile([P, ET], f32)
            for t in range(ET):
                pst = ps_kv.tile([P, 1], f32, tag="ps", name=f"spread{t}")
                nc.tensor.matmul(pst, sums_sb[0:1, t * P:(t + 1) * P], one_sp,
                                 start=True, stop=True)
                nc.vector.tensor_copy(out=rsum[:, t:t + 1], in_=pst)
            nc.vector.reciprocal(out=rsum, in_=rsum)

        # qT + phase-2 pools freed; aT stays resident in SBUF

        # ------------- Phase 3: O, residual, LN1, transpose -------------
        # creation order matters: each pool lands on the phase-2 ranges that
        # die earliest (x->qT, xT->st, wo->st/kc, bc/sq->kc), so their DMAs
        # overlap the tail of the attention phase
        x_pool = ctx.enter_context(tc.tile_pool(name="x_pool", bufs=1))
        xT_pool = ctx.enter_context(tc.tile_pool(name="xT_pool", bufs=1))
        x_sb = x_pool.tile([P, ET, E], f32)   # [q(8x128), e]
        xT = xT_pool.tile([P, ET, NQ], f8)    # [e, q] fp8 for DoubleRow FF1

        wo_pool = ctx.enter_context(tc.tile_pool(name="wo", bufs=1))
        bc_pool = ctx.enter_context(tc.tile_pool(name="bc_pool", bufs=1))
        sq2_pool = ctx.enter_context(tc.tile_pool(name="sq2", bufs=2))
        g1_bc = bcast(bc_pool, g1, E)
        be1_bc = bcast(bc_pool, be1, E)
        ident = consts.tile([P, P], f32)
        make_identity(nc, ident)

        with ExitStack() as ph3:
            ps_o = ph3.enter_context(tc.tile_pool(name="ps_o", bufs=4, space="PSUM"))
            ps_t = ph3.enter_context(tc.tile_pool(name="ps_t", bufs=4, space="PSUM"))

            wo_sb = wo_pool.tile([P, ET, E], f32)
            for e_t in range(ET):
                nc.sync.dma_start(out=_mm(wo_sb[:, e_t, :]),
                                  in_=_mm(_pt(Wo)[:, e_t, :]))

            for q_t in range(ET):
                sq = sq2_pool.tile([P, E], f32, tag="sq")
                nc.sync.dma_start(out=sq, in_=srcq[q_t * P:(q_t + 1) * P, :])
                for eo in range(2):
                    ps = ps_o.tile([P, 512], f32, tag="ps")
                    for e_t in range(ET):
                        nc.tensor.matmul(ps, _mm(aT[:, e_t, q_t * P:(q_t + 1) * P]),
                                         _mm(wo_sb[:, e_t, eo * 512:(eo + 1) * 512]),
                                         start=(e_t == 0), stop=(e_t == ET - 1))
                    # x = O*rsum + (srcq + bo_eff)  (bo folded into srcq on host)
                    nc.vector.scalar_tensor_tensor(
                        out=x_sb[:, q_t, eo * 512:(eo + 1) * 512],
                        in0=ps, scalar=rsum[:, q_t:q_t + 1],
                        in1=sq[:, eo * 512:(eo + 1) * 512],
                        op0=MULT, op1=ADD)
                xt_row = x_sb[:, q_t, :]
                layernorm_inplace(xt_row, g1_bc, be1_bc)
                for e_t in range(ET):
                    pst = ps_t.tile([P, P], f32, tag="ps")
                    nc.tensor.transpose(pst, x_sb[:, q_t, e_t * P:(e_t + 1) * P], ident)
                    nc.scalar.activation(out=xT[:, e_t, q_t * P:(q_t + 1) * P],
                                         in_=pst, func=COPY)

        stA.close()  # aT freed

        # ------------- Phase 4: feedforward + LN2 -------------
        # f-chunk PAIRS: one FF2 PSUM group spans 8 f-tiles (half the
        # drains); b2 enters the last group as a rank-1 ones^T (x) b2
        # matmul; LN2 + output DMA folded per q-tile into the last pair
        g2_bc = bcast(bc_pool, g2, E)
        be2_bc = bcast(bc_pool, be2, E)
        with ExitStack() as ph4:
            w1_pool = ph4.enter_context(tc.tile_pool(name="w1p", bufs=2))
            w2_pool = ph4.enter_context(tc.tile_pool(name="w2p", bufs=3))
            hc_pool = ph4.enter_context(tc.tile_pool(name="hc", bufs=2))
            ps_h = ph4.enter_context(tc.tile_pool(name="ps_h", bufs=3, space="PSUM"))
            ps_f = ph4.enter_context(tc.tile_pool(name="ps_f", bufs=5, space="PSUM"))

            DR = mybir.MatmulPerfMode.DoubleRow
            NPAIR = FT // FCH // 2
            for fp in range(NPAIR):
                last = fp == NPAIR - 1
                hts, w2s = [], []
                for j in range(2):
                    fc = 2 * fp + j
                    w1c = w1_pool.tile([P, ET, FCH * P], f8, tag="w1", name=f"w1c{fc}")
                    nc.sync.dma_start(
                        out=w1c,
                        in_=_pt(W1)[:, :, fc * FCH * P:(fc + 1) * FCH * P])
                    hTc = hc_pool.tile([P, FCH, NQ], f8, tag="hc", name=f"hc{fc}")
                    hts.append(hTc)
                    flqs = ([(fl, qs) for qs in range(QS) for fl in range(FCH)]
                            if fc == 0 else
                            [(fl, qs) for fl in range(FCH) for qs in range(QS)])
                    for fl, qs in flqs:
                        f_t = fc * FCH + fl
                        if True:
                            ps = ps_h.tile([P, 512], f32, tag="ps")
                            for ep in range(ET // 2):
                                nc.tensor.matmul(
                                    ps, w1c[:, 2 * ep:2 * ep + 2, fl * P:(fl + 1) * P],
                                    xT[:, 2 * ep:2 * ep + 2, qs * 512:(qs + 1) * 512],
                                    start=(ep == 0), stop=(ep == ET // 2 - 1),
                                    perf_mode=DR)
                            # h = relu(z + b1), z = psum/W1SC
                            nc.scalar.activation(
                                out=hTc[:, fl, qs * 512:(qs + 1) * 512],
                                in_=ps, func=mybir.ActivationFunctionType.Relu,
                                bias=b1_sb[:, f_t:f_t + 1], scale=1.0 / W1SC)

                    w2c = w2_pool.tile([P, FCH, E], f8, tag="w2", name=f"w2c{fc}")
                    w2s.append(w2c)
                    nc.sync.dma_start(out=w2c,
                                      in_=_pt(W2)[:, fc * FCH:(fc + 1) * FCH, :])

                for q_t in range(ET):
                    for eo in range(2):
                        ps = ps_f.tile([P, 512], f32, tag="ps")
                        for j in range(2):
                            for fh in range(FCH // 2):
                                nc.tensor.matmul(
                                    ps,
                                    hts[j][:, 2 * fh:2 * fh + 2, q_t * P:(q_t + 1) * P],
                                    w2s[j][:, 2 * fh:2 * fh + 2, eo * 512:(eo + 1) * 512],
                                    start=(j == 0 and fh == 0),
                                    stop=(j == 1 and fh == FCH // 2 - 1),
                                    perf_mode=DR)
                        dst = x_sb[:, q_t, eo * 512:(eo + 1) * 512]
                        # x += psum/W2SC  (fused scale+add); b2 is folded
                        # into be1 on the host (x carries it already)
                        nc.vector.scalar_tensor_tensor(
                            out=dst, in0=ps, scalar=invw2, in1=dst,
                            op0=MULT, op1=ADD)
                    if last:
                        row = x_sb[:, q_t, :]
                        layernorm_inplace(row, g2_bc, be2_bc)
                        nc.sync.dma_start(out=out[q_t * P:(q_t + 1) * P, :], in_=row)

    nc.compile()
    return nc


_NC_CACHE = None


def make_in_maps(inputs):
    import ml_dtypes

    src = np.ascontiguousarray(np.asarray(inputs["src"], dtype=np.float32))
    f = lambda n: np.asarray(inputs[n], dtype=np.float32)
    shared = {n: np.ascontiguousarray(f(n))
              for n in ["Wq", "Wk", "Wv", "Wo", "g1", "g2", "be2"]}
    # b2 is added to x before LN2; x = LN1(.)*g1 + be1, so fold b2 into be1
    shared["be1"] = np.ascontiguousarray(f("be1") + f("b2"))
    # FF weights: fp8 e4m3, pre-scaled into fp8-normal range (clip to the
    # TRN e4m3 max of +-240; power-of-2 scales are exact to invert)
    e4 = lambda a: np.clip(a, -240.0, 240.0).astype(ml_dtypes.float8_e4m3fn)
    shared["W1"] = np.ascontiguousarray(e4(f("W1") * W1SC))
    shared["W2"] = np.ascontiguousarray(e4(f("W2") * W2SC))
    # partition-major bias layouts (element i at [i % 128, i // 128])
    shared["bqp"] = np.ascontiguousarray(f("bq").reshape(ET, P).T)
    shared["bkp"] = np.ascontiguousarray(f("bk").reshape(ET, P).T)
    shared["b1p"] = np.ascontiguousarray(f("b1").reshape(FT, P).T)
    # softmax rows sum to 1 -> V-bias contributes bv@Wo to every row;
    # the whole O-bias is folded into the residual srcq on the host
    bo_eff = f("bv") @ shared["Wo"] + f("bo")

    in_maps = []
    for core in range(8):
        b, h = core // 2, core % 2
        src_b = src[b]                        # [2048, 1024]
        # permute context so this core's query half is columns 0..1023
        perm = np.concatenate([src_b[h * NQ:(h + 1) * NQ, :],
                               src_b[(1 - h) * NQ:(2 - h) * NQ, :]])
        srcT = np.ascontiguousarray(perm.T)   # [1024, 2048]
        srcq = np.ascontiguousarray(src_b[h * NQ:(h + 1) * NQ, :] + bo_eff)
        in_maps.append({"srcT": srcT, "srcq": srcq, **shared})
    return in_maps


def gather_out(results):
    out = np.empty((4, S, E), np.float32)
    for core in range(8):
        b, h = core // 2, core % 2
        out[b, h * NQ:(h + 1) * NQ, :] = results[core]["out"]
    return out


def kernel(**inputs):
    global _NC_CACHE
    from concourse.bass_utils import run_bass_kernel_spmd

    in_maps = make_in_maps(inputs)
    if _NC_CACHE is None:
        _NC_CACHE = build_program()
    res = run_bass_kernel_spmd(_NC_CACHE, in_maps, list(range(8)))
    return gather_out(res.results)


if __name__ == "__main__":
    nc = build_program()
    print("build + compile OK")
